# revision 21
# baseline (speedup 1.0000x reference)
"""Trainium2 Bass kernel for nn_MultiHeadAttention_63015760167496.

Computation (see reference): qkv = x @ Wqkv; RoPE on q,k; causal softmax
attention per head; out = einsum('bhts,bshd->bhtd', probs, v);
out.reshape(B,T,C) @ Wout  -- NOTE the reshape is a *head-major* flatten of
[B,H,T,D] into [B,T,C], so final-output row r = h*128 + t//16 depends only on
head h.  Sharding: head-parallel over 8 cores (2 heads/core); every core
computes its two heads end-to-end and produces final-output rows
[256*i, 256*i+256).  Host concatenates -- no collectives.

fp8(e4m3) DoubleRow matmuls (2 packed k-values/PE-cell = 2x bf16
throughput; lhsT [128,2,M] / rhs [128,2,N], contraction 256) carry the QKV
projection and the PV/softmax-denominator matmuls, with a **bf16 island
for t<256 / s<256**: rows with concentrated causal attention (small t) are
the only places where fp8's ~3.6% element noise survives averaging, so
cols 0:256 of the first TQ=512 projection tile and the first two s-chunks
of every attention tile stay bf16 (island cols share one psum accumulation
group with the fp8 cols: the start matmul owns the bank, later start=False
matmuls zero their bytes on first write).  fp8 weights are pre-scaled x16
on the host (Wqkv values ~0.02 sit in the e4m3 subnormal range); the exp
scale SCALE/256 and Wout/16 cancel it exactly.  The out-projection and the
attention score matmuls stay bf16 (scores contract over d=128 only, so
DoubleRow cannot pair them anyway).  y is written bf16 and upcast on host.
Measured error: maxrel 7.5e-3 vs the 2e-2 gate.

Attention uses the S^T layout ([s,t]): softmax denominator via a ones
matmul (partition reduction on the PE), fp8-paired like PV.  Diagonal
128x128 triangle masks multiply the fp8 probs on gpsimd; for fp8 s-chunk
pairs whose two chunks start at different causal columns, the gap columns
of the later chunk are memset to 0 so the shared-width DoubleRow matmul
adds nothing there.

The TRN2 PE clock ramps (0.65 -> 1.2 -> 2.4 GHz) only under *continuous*
load; the schedule interleaves phases with independent tensor work:
    A: qkv(b=0)
    B: attention(b=0) + qkv(b=1)        (interleaved emission)
    C: attention(b=1) + out-proj(b=0)   (interleaved emission)
    D: out-proj(b=1)
RoPE is 3 DVE muls + 1 gpsimd add (sign baked into the sin table).
bf16<->fp8 PE config switches break ldweights pipelining, so attention
drains pop 2+ pending groups at once.  Phase D's Wout reload rides a
second weight pool (wo2) carved from the SBUF the qkv pools free after
phase B, prefetched during phase C.
"""

import math
import sys

for _p in ("/opt/trn_rl_repo", "/root/.axon_site/_ro/trn_rl_repo"):
    if _p not in sys.path:
        sys.path.insert(0, _p)

import numpy as np
import ml_dtypes

import concourse.bass as bass
import concourse.mybir as mybir
import concourse.tile as tile
from concourse import bacc
from concourse.bass_utils import run_bass_kernel_spmd

B, T, C = 2, 2048, 2048
H = 16            # heads total
D = C // H        # 128 head dim
HALF = D // 2     # 64
P = 128
KO = C // P       # 16 contraction chunks
NCORES = 8
HPC = H // NCORES  # 2 heads per core
TQ = 512          # t-tile for qkv projection
TISL = 256        # bf16 island width (t < TISL stays bf16)
NT = T // TQ
TA = 512          # t-tile for attention
NTA = T // TA
SCPT = TA // P    # 4 s-chunks per attention tile
TC_ = 512         # out-projection column tile
NCP = C // TC_
ROPE_BASE = 10000.0
SCALE = 1.0 / math.sqrt(D)
WS = 16.0         # fp8 weight pre-scale (host); exp scale / Wout absorb it

f32 = mybir.dt.float32
bf16 = mybir.dt.bfloat16
f8 = mybir.dt.float8e4
DR = mybir.MatmulPerfMode.DoubleRow


def _chain(*gens):
    for g in gens:
        yield from g


def _scale(gen, f):
    for v in gen:
        yield v * f


def _merge(*gens):
    """Cost-weighted round-robin: always step the generator with the least
    accumulated emitted-tensor-time.  Generators yield ns estimates."""
    acc = [0.0] * len(gens)
    live = list(range(len(gens)))
    while live:
        i = min(live, key=lambda k: acc[k])
        try:
            acc[i] += next(gens[i])
        except StopIteration:
            live.remove(i)


def _build():
    nc = bacc.Bacc("TRN2", target_bir_lowering=False, debug=False,
                   num_devices=NCORES)

    # host-pre-tiled x^T, fp8: xTt[b, ti, p, ko, u] = x[b, ti*TQ+u, ko*128+p]
    xTt = nc.dram_tensor("xTt", [B, NT, P, KO, TQ], f8, kind="ExternalInput")
    # bf16 island copy of the first ti tile (t < 256)
    xI = nc.dram_tensor("xI", [B, P, KO, TISL], bf16, kind="ExternalInput")
    # host-pre-chunked weights (x WS): wq/wk[p, hh, ko, d] = W[ko*128+p, hh*128+d]
    wq = nc.dram_tensor("wq", [P, HPC, KO, D], bf16, kind="ExternalInput")
    wk = nc.dram_tensor("wk", [P, HPC, KO, D], bf16, kind="ExternalInput")
    wv = nc.dram_tensor("wv", [P, KO, HPC * D], bf16, kind="ExternalInput")
    wq8 = nc.dram_tensor("wq8", [P, HPC, KO, D], f8, kind="ExternalInput")
    wk8 = nc.dram_tensor("wk8", [P, HPC, KO, D], f8, kind="ExternalInput")
    wv8 = nc.dram_tensor("wv8", [P, KO, HPC * D], f8, kind="ExternalInput")
    # woutp[cpi, p, j, m] = (Wout/WS)[j*128+p, cpi*TC_+m], bf16
    woutp = nc.dram_tensor("woutp", [NCP, P, KO, TC_], bf16,
                           kind="ExternalInput")
    cs2 = nc.dram_tensor("cs2", [P, T], bf16, kind="ExternalInput")  # [cos;cos]
    # sign-baked sin: snS[0:64] = -sin, snS[64:128] = +sin
    snS = nc.dram_tensor("snS", [P, T], bf16, kind="ExternalInput")
    # tri[s, u] = 1 iff s <= u (valid upper triangle in the S^T layout)
    tri = nc.dram_tensor("tri", [P, P], bf16, kind="ExternalInput")
    tri8 = nc.dram_tensor("tri8", [P, P], f8, kind="ExternalInput")
    y = nc.dram_tensor("y", [B, HPC * D, C], bf16, kind="ExternalOutput")

    with tile.TileContext(nc) as tc:
        with tc.tile_pool(name="const", bufs=1) as cp_, \
             tc.tile_pool(name="qkv", bufs=1) as qp, \
             tc.tile_pool(name="ot", bufs=1) as op_, \
             tc.tile_pool(name="wo", bufs=2) as wop, \
             tc.tile_pool(name="small", bufs=3) as sp, \
             tc.tile_pool(name="pt", bufs=6) as ptp, \
             tc.tile_pool(name="psBsc", bufs=2, space="PSUM") as pssc, \
             tc.tile_pool(name="psBo", bufs=1, space="PSUM") as pso, \
             tc.tile_pool(name="psA", bufs=2, space="PSUM") as psa, \
             tc.tile_pool(name="psBsum", bufs=1, space="PSUM") as pssum:

            wq_sb = cp_.tile([P, HPC, KO, D], bf16, tag="wq")
            wk_sb = cp_.tile([P, HPC, KO, D], bf16, tag="wk")
            wv_sb = cp_.tile([P, KO, HPC * D], bf16, tag="wv")
            wq8_sb = cp_.tile([P, HPC, KO, D], f8, tag="wq8")
            wk8_sb = cp_.tile([P, HPC, KO, D], f8, tag="wk8")
            wv8_sb = cp_.tile([P, KO, HPC * D], f8, tag="wv8")
            # first q matmuls need only wq[:, 0]; split the DMA so they
            # start sooner.  cs/sn head chunks come first: RoPE(ti=0) gates
            # the psum-accumulator recycling.
            nc.sync.dma_start(wq_sb[:, 0], wq.ap()[:, 0])
            cs_sb = cp_.tile([P, T], bf16, tag="cs")
            sn_sb = cp_.tile([P, T], bf16, tag="sn")
            tri_sb = cp_.tile([P, P], bf16, tag="tri")
            tri8_sb = cp_.tile([P, P], f8, tag="tri8")
            # ones *matrix* stationary for the denominator matmuls (bf16 for
            # the island chunks, fp8 k-pair for the DoubleRow chunks): cost ~
            # moving size, keeps the PE tile config at (128,128), and
            # broadcasts the sums to all partitions so normalization is a
            # plain elementwise multiply.
            # PE warmup: the clock ramp (0.65->2.4GHz) needs sustained
            # activity; burn cheap dummy matmuls while the first DMAs land.
            # gpsimd does the memset so the chain doesn't wait on DVE init.
            warm_sb = cp_.tile([P, TA], bf16, tag="warm_sb")
            warm_st = cp_.tile([P, 1], bf16, tag="warm_st")
            nc.gpsimd.memset(warm_st[:], 0.0)
            nc.gpsimd.memset(warm_sb[:], 0.0)
            ps_warm = pssc.tile([P, TA], f32, tag="sc", name="warm")
            for wi in range(10):
                nc.tensor.matmul(ps_warm[0:1, :], warm_st[:], warm_sb[:],
                                 start=True, stop=True)

            ones_f32 = cp_.tile([P, P], f32, tag="ones_f32")
            nc.vector.memset(ones_f32[:], 1.0)
            ones_mat = cp_.tile([P, P], bf16, tag="ones_mat")
            nc.vector.tensor_copy(ones_mat[:], ones_f32[:])
            ones8 = cp_.tile([P, 2, P], f8, tag="ones8")
            nc.vector.memset(ones8[:], 1.0)

            # persistent attention outputs O^T per (b, local head): [d, t]
            oT = [[op_.tile([P, T], bf16, tag=f"oT{b}{hh}", name=f"oT{b}{hh}")
                   for hh in range(HPC)] for b in range(B)]
            qT = [[qp.tile([P, T], bf16, tag=f"qT{b}{hh}", name=f"qT{b}{hh}")
                   for hh in range(HPC)] for b in range(B)]
            kT = [[qp.tile([P, T], bf16, tag=f"kT{b}{hh}", name=f"kT{b}{hh}")
                   for hh in range(HPC)] for b in range(B)]
            # v chunks: s-chunks 0..1 bf16 (island), all chunks 2+ fp8
            vt16 = [[qp.tile([P, 1, D], bf16, tag=f"v16{b}{hh}",
                             name=f"v16{b}{hh}")
                     for hh in range(HPC)] for b in range(B)]
            vt8 = [[qp.tile([P, T // P, D], f8, tag=f"v8{b}{hh}",
                            name=f"v8{b}{hh}")
                    for hh in range(HPC)] for b in range(B)]

            def gen_qkv(b, xp, xpi, psa, rp, state, tis=None):
                for ti in (range(NT) if tis is None else tis):
                    sl = slice(ti * TQ, (ti + 1) * TQ)
                    first_tile = (b == 0 and ti == 0)
                    island = (ti == 0)
                    xi = None
                    if island:
                        if state.get("nextI") is not None:
                            xi, xt = state.pop("nextI")
                        else:
                            xi = xpi.tile([P, KO, TISL], bf16, tag="xti",
                                          name=f"xti{b}")
                            xt = xpi.tile([P, KO, TISL], f8, tag="xt0",
                                          name=f"xt0{b}")
                            if not first_tile:
                                nc.sync.dma_start(xi[:], xI.ap()[b])
                                nc.sync.dma_start(
                                    xt[:], xTt.ap()[b, 0, :, :, TISL:TQ])
                    elif state.get("next") is not None:
                        xt = state.pop("next")
                    else:
                        xt = xp.tile([P, KO, TQ], f8, tag="xt",
                                     name=f"xt{b}_{ti}")
                        nc.sync.dma_start(xt[:], xTt.ap()[b, ti])

                    if not first_tile:
                        # prefetch the NEXT section's x right away: the DMA
                        # queues are idle at a section boundary, and the full
                        # section (~10us of PE work) hides the transfer.
                        if ti + 1 < NT:
                            if (b, ti + 1) > state.get("pref", (-1, -1)):
                                xtn = xp.tile([P, KO, TQ], f8, tag="xt",
                                              name=f"xt{b}_{ti + 1}")
                                nc.sync.dma_start(xtn[:], xTt.ap()[b, ti + 1])
                                state["next"] = xtn
                                state["pref"] = (b, ti + 1)
                        elif b + 1 < B and (b + 1, 0) > state.get(
                                "pref", (-1, -1)):
                            xi2 = xpi.tile([P, KO, TISL], bf16, tag="xti",
                                           name=f"xti{b + 1}")
                            xt2 = xpi.tile([P, KO, TISL], f8, tag="xt0",
                                           name=f"xt0{b + 1}")
                            nc.sync.dma_start(xi2[:], xI.ap()[b + 1])
                            nc.sync.dma_start(
                                xt2[:], xTt.ap()[b + 1, 0, :, :, TISL:TQ])
                            state["nextI"] = (xi2, xt2)
                            state["pref"] = (b + 1, 0)

                    cs = cs_sb[:, sl]
                    sn = sn_sb[:, sl]

                    def qkmm(w_sb, w8_sb, hh, split_dma=False, csn_after=None):
                        ps = psa.tile([P, TQ], f32, tag="acc",
                                      name=f"acc{b}_{ti}_{hh}")
                        if island:
                            # ONE accumulation group per psum bank: bf16
                            # island cols 0:TISL carry the start flag; the
                            # fp8 pair matmuls on cols TISL:TQ zero their
                            # bytes on first write within the started bank.
                            for ko in range(KO):
                                if split_dma and ko % 4 == 0:
                                    kos = slice(ko, ko + 4)
                                    nc.sync.dma_start(xi[:, kos],
                                                      xI.ap()[b, :, kos])
                                    if ko == 4 and csn_after:
                                        nc.sync.dma_start(cs_sb[:, 0:TQ],
                                                          cs2.ap()[:, 0:TQ])
                                        nc.sync.dma_start(sn_sb[:, 0:TQ],
                                                          snS.ap()[:, 0:TQ])
                                nc.tensor.matmul(ps[:, 0:TISL],
                                                 w_sb[:, hh, ko, :],
                                                 xi[:, ko, :],
                                                 start=(ko == 0), stop=False)
                            if split_dma:
                                nc.sync.dma_start(
                                    xt[:], xTt.ap()[b, 0, :, :, TISL:TQ])
                            for kp in range(KO // 2):
                                nc.tensor.matmul(
                                    ps[:, TISL:TQ],
                                    w8_sb[:, hh, 2 * kp:2 * kp + 2, :],
                                    xt[:, 2 * kp:2 * kp + 2, :],
                                    start=False, stop=(kp == KO // 2 - 1),
                                    perf_mode=DR)
                        else:
                            for kp in range(KO // 2):
                                nc.tensor.matmul(
                                    ps[:], w8_sb[:, hh, 2 * kp:2 * kp + 2, :],
                                    xt[:, 2 * kp:2 * kp + 2, :],
                                    start=(kp == 0), stop=(kp == KO // 2 - 1),
                                    perf_mode=DR)
                        return ps

                    def rope(ps, dst):
                        # tcos = ps * [cos;cos]; tsw pre-swaps halves with
                        # the sign baked into snS (rows 0:64 hold -sin), so
                        # ONE full-width gpsimd add finishes the rotation.
                        tcos = rp.tile([P, TQ], bf16, tag="tcos")
                        tsw = rp.tile([P, TQ], bf16, tag="tsw")
                        nc.vector.tensor_mul(tcos[:], ps[:], cs)
                        nc.vector.tensor_mul(tsw[0:HALF, :],
                                             ps[HALF:P, :], sn[0:HALF, :])
                        nc.vector.tensor_mul(tsw[HALF:P, :],
                                             ps[0:HALF, :], sn[HALF:P, :])
                        nc.gpsimd.tensor_add(dst[:, sl], tcos[:], tsw[:])

                    if first_tile:
                        # q accums first (need only wq + xI quarters); the
                        # fp8 half of the island needs wq8[:, 0] -- tiny DMA
                        # issued up front.  Stagger the rest behind.
                        nc.sync.dma_start(wq8_sb[:, 0], wq8.ap()[:, 0])
                        ps0 = qkmm(wq_sb, wq8_sb, 0, split_dma=True,
                                   csn_after=1)
                        nc.sync.dma_start(wq_sb[:, 1], wq.ap()[:, 1])
                        nc.sync.dma_start(wq8_sb[:, 1], wq8.ap()[:, 1])
                        nc.sync.dma_start(wk_sb[:], wk.ap())
                        nc.sync.dma_start(wk8_sb[:], wk8.ap())
                        yield 2600.0
                        ps1 = qkmm(wq_sb, wq8_sb, 1)
                        nc.sync.dma_start(wv_sb[:], wv.ap())
                        nc.sync.dma_start(wv8_sb[:], wv8.ap())
                        xtn = xp.tile([P, KO, TQ], f8, tag="xt",
                                      name=f"xt{b}_1")
                        nc.sync.dma_start(xtn[:], xTt.ap()[b, 1])
                        state["next"] = xtn
                        state["pref"] = (b, 1)
                        rope(ps0, qT[b][0])
                        yield 2600.0
                        psk = qkmm(wk_sb, wk8_sb, 0)
                        nc.sync.dma_start(cs_sb[:, TQ:3 * TQ],
                                          cs2.ap()[:, TQ:3 * TQ])
                        nc.sync.dma_start(sn_sb[:, TQ:3 * TQ],
                                          snS.ap()[:, TQ:3 * TQ])
                        rope(ps1, qT[b][1])
                        rope(psk, kT[b][0])
                        yield 2600.0
                        psk = qkmm(wk_sb, wk8_sb, 1)
                        rope(psk, kT[b][1])
                        yield 2600.0
                    else:
                        for w_sb, w8_sb, dsts in ((wq_sb, wq8_sb, qT[b]),
                                                  (wk_sb, wk8_sb, kT[b])):
                            for hh in range(HPC):
                                rope(qkmm(w_sb, w8_sb, hh), dsts[hh])
                                yield 2600.0 if island else 1707.0
                    if b == 0 and ti == 1:
                        nc.sync.dma_start(cs_sb[:, 3 * TQ:],
                                          cs2.ap()[:, 3 * TQ:])
                        nc.sync.dma_start(sn_sb[:, 3 * TQ:],
                                          snS.ap()[:, 3 * TQ:])
                        nc.sync.dma_start(tri_sb[:], tri.ap())
                        nc.sync.dma_start(tri8_sb[:], tri8.ap())
                    # prefetch next x tiles before the v-section so their
                    # DMAs get ahead of lower-priority queue entries
                    if ti + 1 < NT:
                        if (b, ti + 1) > state.get("pref", (-1, -1)):
                            xtn = xp.tile([P, KO, TQ], f8, tag="xt",
                                          name=f"xt{b}_{ti + 1}")
                            nc.sync.dma_start(xtn[:], xTt.ap()[b, ti + 1])
                            state["next"] = xtn
                            state["pref"] = (b, ti + 1)
                    elif b + 1 < B and (b + 1, 0) > state.get("pref",
                                                              (-1, -1)):
                        xi2 = xpi.tile([P, KO, TISL], bf16, tag="xti",
                                       name=f"xti{b + 1}")
                        xt2 = xpi.tile([P, KO, TISL], f8, tag="xt0",
                                       name=f"xt0{b + 1}")
                        nc.sync.dma_start(xi2[:], xI.ap()[b + 1])
                        nc.sync.dma_start(
                            xt2[:], xTt.ap()[b + 1, 0, :, :, TISL:TQ])
                        state["nextI"] = (xi2, xt2)
                        state["pref"] = (b + 1, 0)
                    for sub in range(TQ // P):
                        tci = ti * (TQ // P) + sub
                        psvt = psa.tile([P, TQ], f32, tag="acc",
                                        name=f"accv{b}_{ti}_{sub}")
                        psv = psvt[:, 0:HPC * D]
                        if tci < 2:
                            for ko in range(KO):
                                nc.tensor.matmul(
                                    psv, xi[:, ko, sub * P:(sub + 1) * P],
                                    wv_sb[:, ko, :],
                                    start=(ko == 0), stop=(ko == KO - 1))
                        elif island:
                            for kp in range(KO // 2):
                                nc.tensor.matmul(
                                    psv,
                                    xt[:, 2 * kp:2 * kp + 2,
                                       (sub - 2) * P:(sub - 1) * P],
                                    wv8_sb[:, 2 * kp:2 * kp + 2, :],
                                    start=(kp == 0),
                                    stop=(kp == KO // 2 - 1),
                                    perf_mode=DR)
                        else:
                            for kp in range(KO // 2):
                                nc.tensor.matmul(
                                    psv,
                                    xt[:, 2 * kp:2 * kp + 2,
                                       sub * P:(sub + 1) * P],
                                    wv8_sb[:, 2 * kp:2 * kp + 2, :],
                                    start=(kp == 0),
                                    stop=(kp == KO // 2 - 1),
                                    perf_mode=DR)
                        for hh in range(HPC):
                            if tci < 1:
                                nc.vector.tensor_copy(
                                    vt16[b][hh][:, tci, :],
                                    psv[:, hh * D:(hh + 1) * D])
                            else:
                                nc.vector.tensor_copy(
                                    vt8[b][hh][:, tci, :],
                                    psv[:, hh * D:(hh + 1) * D])
                        yield 1707.0 if tci < 2 else 853.0

            def gen_attn(b):
                # Both heads interleaved; o/sum matmuls trail score/exp so
                # the PE isn't chained to the Exp latency.  Mask/normalize
                # run on gpsimd.  s-chunks 0,1 are the bf16 island; chunks
                # 2..smax go in fp8 DoubleRow pairs.
                for ta in range(NTA):
                    ps_o = [pso.tile([P, TA], f32, tag=f"o{hh}",
                                     name=f"o{b}_{ta}_{hh}")
                            for hh in range(HPC)]
                    ps_sum = pssum.tile([P, HPC, TA], f32, tag="sum",
                                        name=f"sum{b}_{ta}")
                    smax = (ta + 1) * SCPT - 1
                    # schedule: ('b16', s) singles for s=0,1; ('f8', s)
                    # pairs (s, s+1) for s=2,4,..,smax-1
                    sched = [("b16", 0)] + \
                        [("f8", s) for s in range(1, smax - 1, 2)] + \
                        [("f8s", smax)]
                    pending = []

                    def drain(n, _p=pending, _o=ps_o, _s=ps_sum):
                        while len(_p) > n:
                            kind, s, w, pt2, first, last = _p.pop(0)
                            if kind == "b16":
                                for hh in range(HPC):
                                    nc.tensor.matmul(_o[hh][:, w],
                                                     vt16[b][hh][:, s, :],
                                                     pt2[:, 0, hh, w],
                                                     start=first, stop=last)
                                    nc.tensor.matmul(_s[:, hh, w],
                                                     ones_mat[:],
                                                     pt2[:, 0, hh, w],
                                                     start=first, stop=last)
                            elif kind == "f8s":
                                for hh in range(HPC):
                                    nc.tensor.matmul(_o[hh][:, w],
                                                     vt8[b][hh][:, s, :],
                                                     pt2[:, 0, hh, w],
                                                     start=first, stop=last)
                                    nc.tensor.matmul(_s[:, hh, w],
                                                     ones8[:, 0, :],
                                                     pt2[:, 0, hh, w],
                                                     start=first, stop=last)
                            else:
                                for hh in range(HPC):
                                    nc.tensor.matmul(
                                        _o[hh][:, w],
                                        vt8[b][hh][:, s:s + 2, :],
                                        pt2[:, :, hh, w],
                                        start=first, stop=last, perf_mode=DR)
                                    nc.tensor.matmul(
                                        _s[:, hh, w], ones8[:],
                                        pt2[:, :, hh, w],
                                        start=first, stop=last, perf_mode=DR)

                    for gi, (kind, s) in enumerate(sched):
                        glen = 2 if kind == "f8" else 1
                        j0 = s - ta * SCPT
                        w0g = P * max(j0, 0)
                        wg = slice(w0g, TA)
                        # f8s singles borrow the f8 pair ring (slot 0 only)
                        pt2 = ptp.tile([P, 1 if kind == "b16" else 2,
                                        HPC, TA],
                                       bf16 if kind == "b16" else f8,
                                       tag="ptb16" if kind == "b16"
                                       else "ptf8",
                                       name=f"pt{b}_{ta}_{s}")
                        cost = 0.0
                        for idx in range(glen):
                            sc_ = s + idx
                            j = sc_ - ta * SCPT
                            w0s = P * max(j, 0)
                            ws = slice(w0s, TA)
                            qsl = slice(ta * TA + w0s, (ta + 1) * TA)
                            if idx == 1 and w0s > w0g:
                                # DoubleRow pair shares width w0g; zero the
                                # later chunk's not-yet-valid columns
                                nc.gpsimd.memset(
                                    pt2[:, 1, :, w0g:w0s], 0.0)
                            for hh in range(HPC):
                                ps_sc = pssc.tile(
                                    [P, TA], f32, tag="sc",
                                    name=f"sc{b}_{ta}_{sc_}_{hh}")
                                nc.tensor.matmul(
                                    ps_sc[:, ws],
                                    kT[b][hh][:, sc_ * P:(sc_ + 1) * P],
                                    qT[b][hh][:, qsl],
                                    start=True, stop=True)
                                nc.scalar.activation(
                                    pt2[:, idx, hh, ws], ps_sc[:, ws],
                                    mybir.ActivationFunctionType.Exp,
                                    scale=SCALE / (WS * WS))
                                if j >= 0:  # mask the 128x128 triangle
                                    trm = tri_sb if kind == "b16" else tri8_sb
                                    nc.gpsimd.tensor_mul(
                                        pt2[:, idx, hh, w0s:w0s + P],
                                        pt2[:, idx, hh, w0s:w0s + P],
                                        trm[:])
                                cost += (TA - w0s) * 0.42
                        pending.append((kind, s, wg, pt2, gi == 0,
                                        gi == len(sched) - 1))
                        if len(pending) > 3:
                            drain(1)  # pop 2+ at once: fewer PE dtype switches
                        
                        # score (bf16) + pv/sum (fp8 ~half)
                        yield cost * (3.0 if kind == "b16" else 2.0)
                    drain(0)
                    for hh in range(HPC):
                        recf = sp.tile([P, TA], f32, tag="recf")
                        nc.vector.reciprocal_approx_fast(
                            recf[:], ps_sum[:, hh, :])
                        o_sb = sp.tile([P, TA], f32, tag="o_sb")
                        nc.vector.tensor_copy(o_sb[:], ps_o[hh][:])
                        # write oT pre-shuffled for the out-projection:
                        # oT[p, j*128+u] = O^T[p, t=u*16+j]
                        oview = oT[b][hh].rearrange(
                            "p (j u) -> p u j", j=KO)[
                            :, (TA // 16) * ta:(TA // 16) * (ta + 1), :]
                        nc.gpsimd.tensor_mul(
                            oview,
                            o_sb[:].rearrange("p (u j) -> p u j", j=KO),
                            recf[:].rearrange("p (u j) -> p u j", j=KO))
                    yield 500.0

            def wload(b, cpi, pool=None):
                wcp = (pool or wop).tile([P, KO, TC_], bf16, tag="w",
                                         name=f"w{b}_{cpi}")
                nc.sync.dma_start(wcp[:], woutp.ap()[cpi])
                state_w[(b, cpi)] = wcp

            def gen_out(b, psc, cpis=None):
                for cpi in (cpis if cpis is not None else range(NCP)):
                    csl = slice(cpi * TC_, (cpi + 1) * TC_)
                    if (b, cpi) not in state_w:
                        wload(b, cpi)
                    wcp = state_w[(b, cpi)]
                    if b == 0 and cpi == 1:
                        # b=1's first two Wout tiles ride phase C's DMA slack
                        wload(1, 0, state_w["wop2"])
                        wload(1, 1, state_w["wop2"])
                    if b == 1 and cpi == 0:
                        wload(1, 2)
                        wload(1, 3)
                    nb_, ncpi = (b, cpi + 1) if cpi + 1 < NCP else (b + 1, 0)
                    if nb_ < B and (nb_, ncpi) not in state_w:
                        wload(nb_, ncpi)  # 1-ahead prefetch (wo ring: 2 bufs)
                    for hh in range(HPC):
                        psy = psc.tile([P, TC_], f32, tag="acc",
                                       name=f"y{b}_{cpi}_{hh}")
                        for j in range(KO):
                            nc.tensor.matmul(psy[:],
                                             oT[b][hh][:, j * P:(j + 1) * P],
                                             wcp[:, j, :],
                                             start=(j == 0),
                                             stop=(j == KO - 1))
                        ysb = sp.tile([P, TC_], bf16, tag="ysb")
                        if b == 1:
                            nc.scalar.copy(ysb[:], psy[:])
                        else:
                            nc.vector.tensor_copy(ysb[:], psy[:])
                        nc.sync.dma_start(
                            y.ap()[b, hh * D:(hh + 1) * D, csl], ysb[:])
                        yield 6800.0 if b == 0 else 3414.0

            state_w = {}
            qstate = {}
            with tc.tile_pool(name="xt", bufs=2) as xp, \
                 tc.tile_pool(name="xti", bufs=1) as xpi, \
                 tc.tile_pool(name="rope", bufs=2) as rp:
                # phase A: qkv(b=0) alone
                for _ in gen_qkv(0, xp, xpi, psa, rp, qstate):
                    pass
                # phase B: attention(b=0) interleaved with qkv(b=1)
                _merge(gen_attn(0), gen_qkv(1, xp, xpi, psa, rp, qstate))
                wload(0, 0)
                wload(0, 1)
            # SBUF freed by xt/xti/rope is reused to double-buffer the b=1
            # Wout tiles during phase C, so phase D never waits on DMA.
            with tc.tile_pool(name="wo2", bufs=2) as wop2:
                state_w["wop2"] = wop2
                # phase C: attention(b=1) interleaved with out-proj(b=0);
                # the last b=0 column tile bridges the C->D transition
                # while oT[b=1] finishes normalizing.
                _merge(gen_attn(1), gen_out(0, psa, range(NCP - 1)))
                for _ in gen_out(0, psa, range(NCP - 1, NCP)):
                    pass
                # phase D: out-proj(b=1)
                for _ in gen_out(1, psa):
                    pass

    nc.compile()
    return nc


_NC = None


def _get_nc():
    global _NC
    if _NC is None:
        _NC = _build()
    return _NC


def _host_tables():
    pos = np.arange(T, dtype=np.float32)[:, None]
    div = np.exp(np.arange(0, 2 * HALF, 2, dtype=np.float32)
                 * np.float32(-math.log(ROPE_BASE) / (2 * HALF)))
    ang = pos * div[None, :]
    cosv = np.cos(ang).astype(np.float32)   # [T, HALF]
    sinv = np.sin(ang).astype(np.float32)
    cosT = np.ascontiguousarray(cosv.T)     # [HALF, T]
    sinT = np.ascontiguousarray(sinv.T)
    cs2 = np.ascontiguousarray(np.concatenate([cosT, cosT], axis=0)
                           .astype(ml_dtypes.bfloat16))  # [P, T]
    snS = np.ascontiguousarray(np.concatenate([-sinT, sinT], axis=0)
                               .astype(ml_dtypes.bfloat16))  # [P, T]
    # triangle mask tri[s, u] = 1 iff s <= u
    uu = np.arange(P)[None, :]
    ss = np.arange(P)[:, None]
    trim = (ss <= uu).astype(ml_dtypes.bfloat16)
    tri8m = (ss <= uu).astype(ml_dtypes.float8_e4m3)
    return cs2, snS, trim, tri8m


def _make_in_maps(x, Wqkv, Wout):
    x = np.asarray(x, dtype=np.float32)
    Wqkv = np.asarray(Wqkv, dtype=np.float32)
    Wout = np.asarray(Wout, dtype=np.float32)
    assert x.shape == (B, T, C) and Wqkv.shape == (C, 3 * C) \
        and Wout.shape == (C, C)

    cs2, snS, trim, tri8m = _host_tables()
    # xTt[b, ti, p, ko, u] = x[b, ti*TQ+u, ko*128+p]
    xTr = x.reshape(B, NT, TQ, KO, P).transpose(0, 1, 4, 3, 2)
    xTt = np.ascontiguousarray(xTr.astype(ml_dtypes.float8_e4m3))
    xI = np.ascontiguousarray(xTr[:, 0, :, :, 0:TISL]
                              .astype(ml_dtypes.bfloat16))
    # woutp[cpi, p, j, m] = (Wout/WS)[j*128+p, cpi*TC_+m]
    woutp = np.ascontiguousarray(
        (Wout * np.float32(1.0 / WS)).astype(ml_dtypes.bfloat16)
        .reshape(KO, P, NCP, TC_).transpose(2, 1, 0, 3))

    in_maps = []
    for core in range(NCORES):
        h0 = core * HPC
        cols = slice(h0 * D, (h0 + HPC) * D)
        ws16, ws8 = [], []
        for part in range(3):
            w = Wqkv[:, part * C:(part + 1) * C][:, cols] * np.float32(WS)
            if part < 2:  # wq/wk: [P, HPC, KO, D]
                wr = np.ascontiguousarray(
                    w.reshape(KO, P, HPC, D).transpose(1, 2, 0, 3))
            else:         # wv: [P, KO, HPC*D]
                wr = np.ascontiguousarray(
                    w.reshape(KO, P, HPC * D).transpose(1, 0, 2))
            ws16.append(wr.astype(ml_dtypes.bfloat16))
            ws8.append(wr.astype(ml_dtypes.float8_e4m3))
        in_maps.append({
            "xTt": xTt, "xI": xI,
            "wq": ws16[0], "wk": ws16[1], "wv": ws16[2],
            "wq8": ws8[0], "wk8": ws8[1], "wv8": ws8[2],
            "woutp": woutp,
            "cs2": cs2, "snS": snS, "tri": trim, "tri8": tri8m,
        })
    return in_maps


def _run(x, Wqkv, Wout, trace=False):
    nc = _get_nc()
    in_maps = _make_in_maps(x, Wqkv, Wout)
    res = run_bass_kernel_spmd(nc, in_maps, core_ids=list(range(NCORES)),
                               trace=trace)
    out = np.empty((B, T, C), dtype=np.float32)
    for core in range(NCORES):
        out[:, core * HPC * D:(core + 1) * HPC * D, :] = \
            res.results[core]["y"].astype(np.float32)
    return out, res


def kernel(x, Wqkv, Wout):
    out, _ = _run(x, Wqkv, Wout)
    return out


# revision 22
# speedup vs baseline: 1.0193x; 1.0193x over previous
"""Trainium2 Bass kernel for nn_MultiHeadAttention_63015760167496.

Computation (see reference): qkv = x @ Wqkv; RoPE on q,k; causal softmax
attention per head; out = einsum('bhts,bshd->bhtd', probs, v);
out.reshape(B,T,C) @ Wout  -- NOTE the reshape is a *head-major* flatten of
[B,H,T,D] into [B,T,C], so final-output row r = h*128 + t//16 depends only on
head h.  Sharding: head-parallel over 8 cores (2 heads/core); every core
computes its two heads end-to-end and produces final-output rows
[256*i, 256*i+256).  Host concatenates -- no collectives.

fp8(e4m3) DoubleRow matmuls (2 packed k-values/PE-cell = 2x bf16
throughput; lhsT [128,2,M] / rhs [128,2,N], contraction 256) carry the QKV
projection and the PV/softmax-denominator matmuls, with a **bf16 island
for t<256 / s<256**: rows with concentrated causal attention (small t) are
the only places where fp8's ~3.6% element noise survives averaging, so
cols 0:256 of the first TQ=512 projection tile and the first two s-chunks
of every attention tile stay bf16 (island cols share one psum accumulation
group with the fp8 cols: the start matmul owns the bank, later start=False
matmuls zero their bytes on first write).  fp8 weights are pre-scaled x16
on the host (Wqkv values ~0.02 sit in the e4m3 subnormal range); the exp
scale SCALE/256 and Wout/16 cancel it exactly.  The out-projection and the
attention score matmuls stay bf16 (scores contract over d=128 only, so
DoubleRow cannot pair them anyway).  y is written bf16 and upcast on host.
Measured error: maxrel 7.5e-3 vs the 2e-2 gate.

Attention uses the S^T layout ([s,t]): softmax denominator via a ones
matmul (partition reduction on the PE), fp8-paired like PV.  Diagonal
128x128 triangle masks multiply the fp8 probs on gpsimd; for fp8 s-chunk
pairs whose two chunks start at different causal columns, the gap columns
of the later chunk are memset to 0 so the shared-width DoubleRow matmul
adds nothing there.

The TRN2 PE clock ramps (0.65 -> 1.2 -> 2.4 GHz) only under *continuous*
load; the schedule interleaves phases with independent tensor work:
    A: qkv(b=0)
    B: attention(b=0) + qkv(b=1)        (interleaved emission)
    C: attention(b=1) + out-proj(b=0)   (interleaved emission)
    D: out-proj(b=1)
RoPE is 3 DVE muls + 1 gpsimd add (sign baked into the sin table).
bf16<->fp8 PE config switches break ldweights pipelining, so attention
drains pop 2+ pending groups at once.  Phase D's Wout reload rides a
second weight pool (wo2) carved from the SBUF the qkv pools free after
phase B, prefetched during phase C.
"""

import math
import sys

for _p in ("/opt/trn_rl_repo", "/root/.axon_site/_ro/trn_rl_repo"):
    if _p not in sys.path:
        sys.path.insert(0, _p)

import numpy as np
import ml_dtypes

import concourse.bass as bass
import concourse.mybir as mybir
import concourse.tile as tile
from concourse import bacc
from concourse.bass_utils import run_bass_kernel_spmd

B, T, C = 2, 2048, 2048
H = 16            # heads total
D = C // H        # 128 head dim
HALF = D // 2     # 64
P = 128
KO = C // P       # 16 contraction chunks
NCORES = 8
HPC = H // NCORES  # 2 heads per core
TQ = 512          # t-tile for qkv projection
TISL = 256        # bf16 island width (t < TISL stays bf16)
NT = T // TQ
TA = 512          # t-tile for attention
NTA = T // TA
SCPT = TA // P    # 4 s-chunks per attention tile
TC_ = 512         # out-projection column tile
NCP = C // TC_
ROPE_BASE = 10000.0
SCALE = 1.0 / math.sqrt(D)
WS = 16.0         # fp8 weight pre-scale (host); exp scale / Wout absorb it

f32 = mybir.dt.float32
bf16 = mybir.dt.bfloat16
f8 = mybir.dt.float8e4
DR = mybir.MatmulPerfMode.DoubleRow


def _chain(*gens):
    for g in gens:
        yield from g


def _scale(gen, f):
    for v in gen:
        yield v * f


def _merge(*gens):
    """Cost-weighted round-robin: always step the generator with the least
    accumulated emitted-tensor-time.  Generators yield ns estimates."""
    acc = [0.0] * len(gens)
    live = list(range(len(gens)))
    while live:
        i = min(live, key=lambda k: acc[k])
        try:
            acc[i] += next(gens[i])
        except StopIteration:
            live.remove(i)


def _build():
    nc = bacc.Bacc("TRN2", target_bir_lowering=False, debug=False,
                   num_devices=NCORES)

    # host-pre-tiled x^T, fp8: xTt[b, ti, p, ko, u] = x[b, ti*TQ+u, ko*128+p]
    xTt = nc.dram_tensor("xTt", [B, NT, P, KO, TQ], f8, kind="ExternalInput")
    # bf16 island copy of the first ti tile (t < 256)
    xI = nc.dram_tensor("xI", [B, P, KO, TISL], bf16, kind="ExternalInput")
    # host-pre-chunked weights (x WS): wq/wk[p, hh, ko, d] = W[ko*128+p, hh*128+d]
    wq = nc.dram_tensor("wq", [P, HPC, KO, D], bf16, kind="ExternalInput")
    wk = nc.dram_tensor("wk", [P, HPC, KO, D], bf16, kind="ExternalInput")
    wv = nc.dram_tensor("wv", [P, KO, HPC * D], bf16, kind="ExternalInput")
    wq8 = nc.dram_tensor("wq8", [P, HPC, KO, D], f8, kind="ExternalInput")
    wk8 = nc.dram_tensor("wk8", [P, HPC, KO, D], f8, kind="ExternalInput")
    wv8 = nc.dram_tensor("wv8", [P, KO, HPC * D], f8, kind="ExternalInput")
    # woutp[cpi, p, j, m] = (Wout/WS)[j*128+p, cpi*TC_+m], bf16
    woutp = nc.dram_tensor("woutp", [NCP, P, KO, TC_], bf16,
                           kind="ExternalInput")
    cs2 = nc.dram_tensor("cs2", [P, T], bf16, kind="ExternalInput")  # [cos;cos]
    # sign-baked sin: snS[0:64] = -sin, snS[64:128] = +sin
    snS = nc.dram_tensor("snS", [P, T], bf16, kind="ExternalInput")
    # tri[s, u] = 1 iff s <= u (valid upper triangle in the S^T layout)
    tri = nc.dram_tensor("tri", [P, P], bf16, kind="ExternalInput")
    tri8 = nc.dram_tensor("tri8", [P, P], f8, kind="ExternalInput")
    y = nc.dram_tensor("y", [B, HPC * D, C], bf16, kind="ExternalOutput")

    with tile.TileContext(nc) as tc:
        with tc.tile_pool(name="const", bufs=1) as cp_, \
             tc.tile_pool(name="qkv", bufs=1) as qp, \
             tc.tile_pool(name="ot", bufs=1) as op_, \
             tc.tile_pool(name="wo", bufs=2) as wop, \
             tc.tile_pool(name="small", bufs=3) as sp, \
             tc.tile_pool(name="pt", bufs=6) as ptp, \
             tc.tile_pool(name="psBsc", bufs=2, space="PSUM") as pssc, \
             tc.tile_pool(name="psBo", bufs=1, space="PSUM") as pso, \
             tc.tile_pool(name="psA", bufs=2, space="PSUM") as psa, \
             tc.tile_pool(name="psBsum", bufs=1, space="PSUM") as pssum:

            wq_sb = cp_.tile([P, HPC, KO, D], bf16, tag="wq")
            wk_sb = cp_.tile([P, HPC, KO, D], bf16, tag="wk")
            wv_sb = cp_.tile([P, KO, HPC * D], bf16, tag="wv")
            wq8_sb = cp_.tile([P, HPC, KO, D], f8, tag="wq8")
            wk8_sb = cp_.tile([P, HPC, KO, D], f8, tag="wk8")
            wv8_sb = cp_.tile([P, KO, HPC * D], f8, tag="wv8")
            # first q matmuls need only wq[:, 0]; split the DMA so they
            # start sooner.  cs/sn head chunks come first: RoPE(ti=0) gates
            # the psum-accumulator recycling.
            nc.sync.dma_start(wq_sb[:, 0], wq.ap()[:, 0])
            cs_sb = cp_.tile([P, T], bf16, tag="cs")
            sn_sb = cp_.tile([P, T], bf16, tag="sn")
            tri_sb = cp_.tile([P, P], bf16, tag="tri")
            tri8_sb = cp_.tile([P, P], f8, tag="tri8")
            # ones *matrix* stationary for the denominator matmuls (bf16 for
            # the island chunks, fp8 k-pair for the DoubleRow chunks): cost ~
            # moving size, keeps the PE tile config at (128,128), and
            # broadcasts the sums to all partitions so normalization is a
            # plain elementwise multiply.
            # PE warmup: the clock ramp (0.65->2.4GHz) needs sustained
            # activity; burn cheap dummy matmuls while the first DMAs land.
            # gpsimd does the memset so the chain doesn't wait on DVE init.
            warm_sb = cp_.tile([P, TA], bf16, tag="warm_sb")
            warm_st = cp_.tile([P, 1], bf16, tag="warm_st")
            nc.gpsimd.memset(warm_st[:], 0.0)
            nc.gpsimd.memset(warm_sb[:], 0.0)
            ps_warm = pssc.tile([P, TA], f32, tag="sc", name="warm")
            for wi in range(14):
                nc.tensor.matmul(ps_warm[0:1, :], warm_st[:], warm_sb[:],
                                 start=True, stop=True)

            ones_f32 = cp_.tile([P, P], f32, tag="ones_f32")
            nc.vector.memset(ones_f32[:], 1.0)
            ones_mat = cp_.tile([P, P], bf16, tag="ones_mat")
            nc.vector.tensor_copy(ones_mat[:], ones_f32[:])
            ones8 = cp_.tile([P, 2, P], f8, tag="ones8")
            nc.vector.memset(ones8[:], 1.0)

            # persistent attention outputs O^T per (b, local head): [d, t]
            oT = [[op_.tile([P, T], bf16, tag=f"oT{b}{hh}", name=f"oT{b}{hh}")
                   for hh in range(HPC)] for b in range(B)]
            qT = [[qp.tile([P, T], bf16, tag=f"qT{b}{hh}", name=f"qT{b}{hh}")
                   for hh in range(HPC)] for b in range(B)]
            kT = [[qp.tile([P, T], bf16, tag=f"kT{b}{hh}", name=f"kT{b}{hh}")
                   for hh in range(HPC)] for b in range(B)]
            # v chunks: s-chunks 0..1 bf16 (island), all chunks 2+ fp8
            vt16 = [[qp.tile([P, 1, D], bf16, tag=f"v16{b}{hh}",
                             name=f"v16{b}{hh}")
                     for hh in range(HPC)] for b in range(B)]
            vt8 = [[qp.tile([P, T // P, D], f8, tag=f"v8{b}{hh}",
                            name=f"v8{b}{hh}")
                    for hh in range(HPC)] for b in range(B)]

            def gen_qkv(b, xp, xpi, psa, rp, state, tis=None):
                for ti in (range(NT) if tis is None else tis):
                    sl = slice(ti * TQ, (ti + 1) * TQ)
                    first_tile = (b == 0 and ti == 0)
                    island = (ti == 0)
                    xi = None
                    if island:
                        if state.get("nextI") is not None:
                            xi, xt = state.pop("nextI")
                        else:
                            xi = xpi.tile([P, KO, TISL], bf16, tag="xti",
                                          name=f"xti{b}")
                            xt = xpi.tile([P, KO, TISL], f8, tag="xt0",
                                          name=f"xt0{b}")
                            if not first_tile:
                                nc.sync.dma_start(xi[:], xI.ap()[b])
                                nc.sync.dma_start(
                                    xt[:], xTt.ap()[b, 0, :, :, TISL:TQ])
                    elif state.get("next") is not None:
                        xt = state.pop("next")
                    else:
                        xt = xp.tile([P, KO, TQ], f8, tag="xt",
                                     name=f"xt{b}_{ti}")
                        nc.sync.dma_start(xt[:], xTt.ap()[b, ti])

                    if not first_tile:
                        # prefetch the NEXT section's x right away: the DMA
                        # queues are idle at a section boundary, and the full
                        # section (~10us of PE work) hides the transfer.
                        if ti + 1 < NT:
                            if (b, ti + 1) > state.get("pref", (-1, -1)):
                                xtn = xp.tile([P, KO, TQ], f8, tag="xt",
                                              name=f"xt{b}_{ti + 1}")
                                nc.sync.dma_start(xtn[:], xTt.ap()[b, ti + 1])
                                state["next"] = xtn
                                state["pref"] = (b, ti + 1)
                        elif b + 1 < B and (b + 1, 0) > state.get(
                                "pref", (-1, -1)):
                            xi2 = xpi.tile([P, KO, TISL], bf16, tag="xti",
                                           name=f"xti{b + 1}")
                            xt2 = xpi.tile([P, KO, TISL], f8, tag="xt0",
                                           name=f"xt0{b + 1}")
                            nc.sync.dma_start(xi2[:], xI.ap()[b + 1])
                            nc.sync.dma_start(
                                xt2[:], xTt.ap()[b + 1, 0, :, :, TISL:TQ])
                            state["nextI"] = (xi2, xt2)
                            state["pref"] = (b + 1, 0)

                    cs = cs_sb[:, sl]
                    sn = sn_sb[:, sl]

                    def qkmm(w_sb, w8_sb, hh, split_dma=False, csn_after=None):
                        ps = psa.tile([P, TQ], f32, tag="acc",
                                      name=f"acc{b}_{ti}_{hh}")
                        if island:
                            # ONE accumulation group per psum bank: bf16
                            # island cols 0:TISL carry the start flag; the
                            # fp8 pair matmuls on cols TISL:TQ zero their
                            # bytes on first write within the started bank.
                            for ko in range(KO):
                                if split_dma and ko % 4 == 0:
                                    kos = slice(ko, ko + 4)
                                    nc.sync.dma_start(xi[:, kos],
                                                      xI.ap()[b, :, kos])
                                    if ko == 4 and csn_after:
                                        nc.sync.dma_start(cs_sb[:, 0:TQ],
                                                          cs2.ap()[:, 0:TQ])
                                        nc.sync.dma_start(sn_sb[:, 0:TQ],
                                                          snS.ap()[:, 0:TQ])
                                nc.tensor.matmul(ps[:, 0:TISL],
                                                 w_sb[:, hh, ko, :],
                                                 xi[:, ko, :],
                                                 start=(ko == 0), stop=False)
                            if split_dma:
                                nc.sync.dma_start(
                                    xt[:], xTt.ap()[b, 0, :, :, TISL:TQ])
                            for kp in range(KO // 2):
                                nc.tensor.matmul(
                                    ps[:, TISL:TQ],
                                    w8_sb[:, hh, 2 * kp:2 * kp + 2, :],
                                    xt[:, 2 * kp:2 * kp + 2, :],
                                    start=False, stop=(kp == KO // 2 - 1),
                                    perf_mode=DR)
                        else:
                            for kp in range(KO // 2):
                                nc.tensor.matmul(
                                    ps[:], w8_sb[:, hh, 2 * kp:2 * kp + 2, :],
                                    xt[:, 2 * kp:2 * kp + 2, :],
                                    start=(kp == 0), stop=(kp == KO // 2 - 1),
                                    perf_mode=DR)
                        return ps

                    def rope(ps, dst):
                        # tcos = ps * [cos;cos]; tsw pre-swaps halves with
                        # the sign baked into snS (rows 0:64 hold -sin), so
                        # ONE full-width gpsimd add finishes the rotation.
                        tcos = rp.tile([P, TQ], bf16, tag="tcos")
                        tsw = rp.tile([P, TQ], bf16, tag="tsw")
                        nc.vector.tensor_mul(tcos[:], ps[:], cs)
                        nc.vector.tensor_mul(tsw[0:HALF, :],
                                             ps[HALF:P, :], sn[0:HALF, :])
                        nc.vector.tensor_mul(tsw[HALF:P, :],
                                             ps[0:HALF, :], sn[HALF:P, :])
                        nc.gpsimd.tensor_add(dst[:, sl], tcos[:], tsw[:])

                    if first_tile:
                        # q accums first (need only wq + xI quarters); the
                        # fp8 half of the island needs wq8[:, 0] -- tiny DMA
                        # issued up front.  Stagger the rest behind.
                        nc.sync.dma_start(wq8_sb[:, 0], wq8.ap()[:, 0])
                        ps0 = qkmm(wq_sb, wq8_sb, 0, split_dma=True,
                                   csn_after=1)
                        nc.sync.dma_start(wq_sb[:, 1], wq.ap()[:, 1])
                        nc.sync.dma_start(wq8_sb[:, 1], wq8.ap()[:, 1])
                        nc.sync.dma_start(wk_sb[:], wk.ap())
                        nc.sync.dma_start(wk8_sb[:], wk8.ap())
                        yield 2600.0
                        ps1 = qkmm(wq_sb, wq8_sb, 1)
                        nc.sync.dma_start(wv_sb[:], wv.ap())
                        nc.sync.dma_start(wv8_sb[:], wv8.ap())
                        xtn = xp.tile([P, KO, TQ], f8, tag="xt",
                                      name=f"xt{b}_1")
                        nc.sync.dma_start(xtn[:], xTt.ap()[b, 1])
                        state["next"] = xtn
                        state["pref"] = (b, 1)
                        rope(ps0, qT[b][0])
                        yield 2600.0
                        psk = qkmm(wk_sb, wk8_sb, 0)
                        nc.sync.dma_start(cs_sb[:, TQ:3 * TQ],
                                          cs2.ap()[:, TQ:3 * TQ])
                        nc.sync.dma_start(sn_sb[:, TQ:3 * TQ],
                                          snS.ap()[:, TQ:3 * TQ])
                        rope(ps1, qT[b][1])
                        rope(psk, kT[b][0])
                        yield 2600.0
                        psk = qkmm(wk_sb, wk8_sb, 1)
                        rope(psk, kT[b][1])
                        yield 2600.0
                    else:
                        for w_sb, w8_sb, dsts in ((wq_sb, wq8_sb, qT[b]),
                                                  (wk_sb, wk8_sb, kT[b])):
                            for hh in range(HPC):
                                rope(qkmm(w_sb, w8_sb, hh), dsts[hh])
                                yield 2600.0 if island else 1707.0
                    if b == 0 and ti == 1:
                        nc.sync.dma_start(cs_sb[:, 3 * TQ:],
                                          cs2.ap()[:, 3 * TQ:])
                        nc.sync.dma_start(sn_sb[:, 3 * TQ:],
                                          snS.ap()[:, 3 * TQ:])
                        nc.sync.dma_start(tri_sb[:], tri.ap())
                        nc.sync.dma_start(tri8_sb[:], tri8.ap())
                    # prefetch next x tiles before the v-section so their
                    # DMAs get ahead of lower-priority queue entries
                    if ti + 1 < NT:
                        if (b, ti + 1) > state.get("pref", (-1, -1)):
                            xtn = xp.tile([P, KO, TQ], f8, tag="xt",
                                          name=f"xt{b}_{ti + 1}")
                            nc.sync.dma_start(xtn[:], xTt.ap()[b, ti + 1])
                            state["next"] = xtn
                            state["pref"] = (b, ti + 1)
                    elif b + 1 < B and (b + 1, 0) > state.get("pref",
                                                              (-1, -1)):
                        xi2 = xpi.tile([P, KO, TISL], bf16, tag="xti",
                                       name=f"xti{b + 1}")
                        xt2 = xpi.tile([P, KO, TISL], f8, tag="xt0",
                                       name=f"xt0{b + 1}")
                        nc.sync.dma_start(xi2[:], xI.ap()[b + 1])
                        nc.sync.dma_start(
                            xt2[:], xTt.ap()[b + 1, 0, :, :, TISL:TQ])
                        state["nextI"] = (xi2, xt2)
                        state["pref"] = (b + 1, 0)
                    for sub in range(TQ // P):
                        tci = ti * (TQ // P) + sub
                        psvt = psa.tile([P, TQ], f32, tag="acc",
                                        name=f"accv{b}_{ti}_{sub}")
                        psv = psvt[:, 0:HPC * D]
                        if tci < 2:
                            for ko in range(KO):
                                nc.tensor.matmul(
                                    psv, xi[:, ko, sub * P:(sub + 1) * P],
                                    wv_sb[:, ko, :],
                                    start=(ko == 0), stop=(ko == KO - 1))
                        elif island:
                            for kp in range(KO // 2):
                                nc.tensor.matmul(
                                    psv,
                                    xt[:, 2 * kp:2 * kp + 2,
                                       (sub - 2) * P:(sub - 1) * P],
                                    wv8_sb[:, 2 * kp:2 * kp + 2, :],
                                    start=(kp == 0),
                                    stop=(kp == KO // 2 - 1),
                                    perf_mode=DR)
                        else:
                            for kp in range(KO // 2):
                                nc.tensor.matmul(
                                    psv,
                                    xt[:, 2 * kp:2 * kp + 2,
                                       sub * P:(sub + 1) * P],
                                    wv8_sb[:, 2 * kp:2 * kp + 2, :],
                                    start=(kp == 0),
                                    stop=(kp == KO // 2 - 1),
                                    perf_mode=DR)
                        for hh in range(HPC):
                            if tci < 1:
                                nc.vector.tensor_copy(
                                    vt16[b][hh][:, tci, :],
                                    psv[:, hh * D:(hh + 1) * D])
                            else:
                                nc.vector.tensor_copy(
                                    vt8[b][hh][:, tci, :],
                                    psv[:, hh * D:(hh + 1) * D])
                        yield 1707.0 if tci < 2 else 853.0

            def gen_attn(b):
                # Both heads interleaved; o/sum matmuls trail score/exp so
                # the PE isn't chained to the Exp latency.  Mask/normalize
                # run on gpsimd.  s-chunks 0,1 are the bf16 island; chunks
                # 2..smax go in fp8 DoubleRow pairs.
                for ta in range(NTA):
                    ps_o = [pso.tile([P, TA], f32, tag=f"o{hh}",
                                     name=f"o{b}_{ta}_{hh}")
                            for hh in range(HPC)]
                    ps_sum = pssum.tile([P, HPC, TA], f32, tag="sum",
                                        name=f"sum{b}_{ta}")
                    smax = (ta + 1) * SCPT - 1
                    # schedule: ('b16', s) singles for s=0,1; ('f8', s)
                    # pairs (s, s+1) for s=2,4,..,smax-1
                    sched = [("b16", 0)] + \
                        [("f8", s) for s in range(1, smax - 1, 2)] + \
                        [("f8s", smax)]
                    pending = []

                    def drain(n, _p=pending, _o=ps_o, _s=ps_sum):
                        while len(_p) > n:
                            kind, s, w, pt2, first, last = _p.pop(0)
                            if kind == "b16":
                                for hh in range(HPC):
                                    nc.tensor.matmul(_o[hh][:, w],
                                                     vt16[b][hh][:, s, :],
                                                     pt2[:, 0, hh, w],
                                                     start=first, stop=last)
                                    nc.tensor.matmul(_s[:, hh, w],
                                                     ones_mat[:],
                                                     pt2[:, 0, hh, w],
                                                     start=first, stop=last)
                            elif kind == "f8s":
                                for hh in range(HPC):
                                    nc.tensor.matmul(_o[hh][:, w],
                                                     vt8[b][hh][:, s, :],
                                                     pt2[:, 0, hh, w],
                                                     start=first, stop=last)
                                    nc.tensor.matmul(_s[:, hh, w],
                                                     ones8[:, 0, :],
                                                     pt2[:, 0, hh, w],
                                                     start=first, stop=last)
                            else:
                                for hh in range(HPC):
                                    nc.tensor.matmul(
                                        _o[hh][:, w],
                                        vt8[b][hh][:, s:s + 2, :],
                                        pt2[:, :, hh, w],
                                        start=first, stop=last, perf_mode=DR)
                                    nc.tensor.matmul(
                                        _s[:, hh, w], ones8[:],
                                        pt2[:, :, hh, w],
                                        start=first, stop=last, perf_mode=DR)

                    for gi, (kind, s) in enumerate(sched):
                        glen = 2 if kind == "f8" else 1
                        j0 = s - ta * SCPT
                        w0g = P * max(j0, 0)
                        wg = slice(w0g, TA)
                        # f8s singles borrow the f8 pair ring (slot 0 only)
                        pt2 = ptp.tile([P, 1 if kind == "b16" else 2,
                                        HPC, TA],
                                       bf16 if kind == "b16" else f8,
                                       tag="ptb16" if kind == "b16"
                                       else "ptf8",
                                       name=f"pt{b}_{ta}_{s}")
                        cost = 0.0
                        for idx in range(glen):
                            sc_ = s + idx
                            j = sc_ - ta * SCPT
                            w0s = P * max(j, 0)
                            ws = slice(w0s, TA)
                            qsl = slice(ta * TA + w0s, (ta + 1) * TA)
                            if idx == 1 and w0s > w0g:
                                # DoubleRow pair shares width w0g; zero the
                                # later chunk's not-yet-valid columns
                                nc.gpsimd.memset(
                                    pt2[:, 1, :, w0g:w0s], 0.0)
                            for hh in range(HPC):
                                ps_sc = pssc.tile(
                                    [P, TA], f32, tag="sc",
                                    name=f"sc{b}_{ta}_{sc_}_{hh}")
                                nc.tensor.matmul(
                                    ps_sc[:, ws],
                                    kT[b][hh][:, sc_ * P:(sc_ + 1) * P],
                                    qT[b][hh][:, qsl],
                                    start=True, stop=True)
                                nc.scalar.activation(
                                    pt2[:, idx, hh, ws], ps_sc[:, ws],
                                    mybir.ActivationFunctionType.Exp,
                                    scale=SCALE / (WS * WS))
                                if j >= 0:  # mask the 128x128 triangle
                                    trm = tri_sb if kind == "b16" else tri8_sb
                                    nc.gpsimd.tensor_mul(
                                        pt2[:, idx, hh, w0s:w0s + P],
                                        pt2[:, idx, hh, w0s:w0s + P],
                                        trm[:])
                                cost += (TA - w0s) * 0.42
                        pending.append((kind, s, wg, pt2, gi == 0,
                                        gi == len(sched) - 1))
                        if len(pending) > 3:
                            drain(1)  # pop 2+ at once: fewer PE dtype switches
                        
                        # score (bf16) + pv/sum (fp8 ~half)
                        yield cost * (3.0 if kind == "b16" else 2.0)
                    drain(0)
                    for hh in range(HPC):
                        recf = sp.tile([P, TA], f32, tag="recf")
                        nc.vector.reciprocal_approx_fast(
                            recf[:], ps_sum[:, hh, :])
                        o_sb = sp.tile([P, TA], f32, tag="o_sb")
                        nc.vector.tensor_copy(o_sb[:], ps_o[hh][:])
                        # write oT pre-shuffled for the out-projection:
                        # oT[p, j*128+u] = O^T[p, t=u*16+j]
                        oview = oT[b][hh].rearrange(
                            "p (j u) -> p u j", j=KO)[
                            :, (TA // 16) * ta:(TA // 16) * (ta + 1), :]
                        nc.gpsimd.tensor_mul(
                            oview,
                            o_sb[:].rearrange("p (u j) -> p u j", j=KO),
                            recf[:].rearrange("p (u j) -> p u j", j=KO))
                    yield 500.0

            def wload(b, cpi, pool=None):
                wcp = (pool or wop).tile([P, KO, TC_], bf16, tag="w",
                                         name=f"w{b}_{cpi}")
                nc.sync.dma_start(wcp[:], woutp.ap()[cpi])
                state_w[(b, cpi)] = wcp

            def gen_out(b, psc, cpis=None):
                for cpi in (cpis if cpis is not None else range(NCP)):
                    csl = slice(cpi * TC_, (cpi + 1) * TC_)
                    if (b, cpi) not in state_w:
                        wload(b, cpi)
                    wcp = state_w[(b, cpi)]
                    if b == 0 and cpi == 1:
                        # b=1's first two Wout tiles ride phase C's DMA slack
                        wload(1, 0, state_w["wop2"])
                        wload(1, 1, state_w["wop2"])
                    if b == 1 and cpi == 0:
                        wload(1, 2)
                        wload(1, 3)
                    nb_, ncpi = (b, cpi + 1) if cpi + 1 < NCP else (b + 1, 0)
                    if nb_ < B and (nb_, ncpi) not in state_w:
                        wload(nb_, ncpi)  # 1-ahead prefetch (wo ring: 2 bufs)
                    for hh in range(HPC):
                        psy = psc.tile([P, TC_], f32, tag="acc",
                                       name=f"y{b}_{cpi}_{hh}")
                        for j in range(KO):
                            nc.tensor.matmul(psy[:],
                                             oT[b][hh][:, j * P:(j + 1) * P],
                                             wcp[:, j, :],
                                             start=(j == 0),
                                             stop=(j == KO - 1))
                        ysb = sp.tile([P, TC_], bf16, tag="ysb")
                        if b == 1:
                            nc.scalar.copy(ysb[:], psy[:])
                        else:
                            nc.vector.tensor_copy(ysb[:], psy[:])
                        nc.sync.dma_start(
                            y.ap()[b, hh * D:(hh + 1) * D, csl], ysb[:])
                        yield 6800.0 if b == 0 else 3414.0

            state_w = {}
            qstate = {}
            with tc.tile_pool(name="xt", bufs=2) as xp, \
                 tc.tile_pool(name="xti", bufs=1) as xpi, \
                 tc.tile_pool(name="rope", bufs=2) as rp:
                # phase A: qkv(b=0) alone
                for _ in gen_qkv(0, xp, xpi, psa, rp, qstate):
                    pass
                # phase B: attention(b=0) interleaved with qkv(b=1)
                _merge(gen_attn(0), gen_qkv(1, xp, xpi, psa, rp, qstate))
                wload(0, 0)
                wload(0, 1)
            # SBUF freed by xt/xti/rope is reused to double-buffer the b=1
            # Wout tiles during phase C, so phase D never waits on DMA.
            with tc.tile_pool(name="wo2", bufs=2) as wop2:
                state_w["wop2"] = wop2
                # phase C: attention(b=1) interleaved with out-proj(b=0);
                # the last b=0 column tile bridges the C->D transition
                # while oT[b=1] finishes normalizing.
                _merge(gen_attn(1), gen_out(0, psa, range(NCP - 1)))
                for _ in gen_out(0, psa, range(NCP - 1, NCP)):
                    pass
                # phase D: out-proj(b=1)
                for _ in gen_out(1, psa):
                    pass

    nc.compile()
    return nc


_NC = None


def _get_nc():
    global _NC
    if _NC is None:
        _NC = _build()
    return _NC


def _host_tables():
    pos = np.arange(T, dtype=np.float32)[:, None]
    div = np.exp(np.arange(0, 2 * HALF, 2, dtype=np.float32)
                 * np.float32(-math.log(ROPE_BASE) / (2 * HALF)))
    ang = pos * div[None, :]
    cosv = np.cos(ang).astype(np.float32)   # [T, HALF]
    sinv = np.sin(ang).astype(np.float32)
    cosT = np.ascontiguousarray(cosv.T)     # [HALF, T]
    sinT = np.ascontiguousarray(sinv.T)
    cs2 = np.ascontiguousarray(np.concatenate([cosT, cosT], axis=0)
                           .astype(ml_dtypes.bfloat16))  # [P, T]
    snS = np.ascontiguousarray(np.concatenate([-sinT, sinT], axis=0)
                               .astype(ml_dtypes.bfloat16))  # [P, T]
    # triangle mask tri[s, u] = 1 iff s <= u
    uu = np.arange(P)[None, :]
    ss = np.arange(P)[:, None]
    trim = (ss <= uu).astype(ml_dtypes.bfloat16)
    tri8m = (ss <= uu).astype(ml_dtypes.float8_e4m3)
    return cs2, snS, trim, tri8m


def _make_in_maps(x, Wqkv, Wout):
    x = np.asarray(x, dtype=np.float32)
    Wqkv = np.asarray(Wqkv, dtype=np.float32)
    Wout = np.asarray(Wout, dtype=np.float32)
    assert x.shape == (B, T, C) and Wqkv.shape == (C, 3 * C) \
        and Wout.shape == (C, C)

    cs2, snS, trim, tri8m = _host_tables()
    # xTt[b, ti, p, ko, u] = x[b, ti*TQ+u, ko*128+p]
    xTr = x.reshape(B, NT, TQ, KO, P).transpose(0, 1, 4, 3, 2)
    xTt = np.ascontiguousarray(xTr.astype(ml_dtypes.float8_e4m3))
    xI = np.ascontiguousarray(xTr[:, 0, :, :, 0:TISL]
                              .astype(ml_dtypes.bfloat16))
    # woutp[cpi, p, j, m] = (Wout/WS)[j*128+p, cpi*TC_+m]
    woutp = np.ascontiguousarray(
        (Wout * np.float32(1.0 / WS)).astype(ml_dtypes.bfloat16)
        .reshape(KO, P, NCP, TC_).transpose(2, 1, 0, 3))

    in_maps = []
    for core in range(NCORES):
        h0 = core * HPC
        cols = slice(h0 * D, (h0 + HPC) * D)
        ws16, ws8 = [], []
        for part in range(3):
            w = Wqkv[:, part * C:(part + 1) * C][:, cols] * np.float32(WS)
            if part < 2:  # wq/wk: [P, HPC, KO, D]
                wr = np.ascontiguousarray(
                    w.reshape(KO, P, HPC, D).transpose(1, 2, 0, 3))
            else:         # wv: [P, KO, HPC*D]
                wr = np.ascontiguousarray(
                    w.reshape(KO, P, HPC * D).transpose(1, 0, 2))
            ws16.append(wr.astype(ml_dtypes.bfloat16))
            ws8.append(wr.astype(ml_dtypes.float8_e4m3))
        in_maps.append({
            "xTt": xTt, "xI": xI,
            "wq": ws16[0], "wk": ws16[1], "wv": ws16[2],
            "wq8": ws8[0], "wk8": ws8[1], "wv8": ws8[2],
            "woutp": woutp,
            "cs2": cs2, "snS": snS, "tri": trim, "tri8": tri8m,
        })
    return in_maps


def _run(x, Wqkv, Wout, trace=False):
    nc = _get_nc()
    in_maps = _make_in_maps(x, Wqkv, Wout)
    res = run_bass_kernel_spmd(nc, in_maps, core_ids=list(range(NCORES)),
                               trace=trace)
    out = np.empty((B, T, C), dtype=np.float32)
    for core in range(NCORES):
        out[:, core * HPC * D:(core + 1) * HPC * D, :] = \
            res.results[core]["y"].astype(np.float32)
    return out, res


def kernel(x, Wqkv, Wout):
    out, _ = _run(x, Wqkv, Wout)
    return out


# revision 25
# speedup vs baseline: 1.0199x; 1.0006x over previous
"""Trainium2 Bass kernel for nn_MultiHeadAttention_63015760167496.

Computation (see reference): qkv = x @ Wqkv; RoPE on q,k; causal softmax
attention per head; out = einsum('bhts,bshd->bhtd', probs, v);
out.reshape(B,T,C) @ Wout  -- NOTE the reshape is a *head-major* flatten of
[B,H,T,D] into [B,T,C], so final-output row r = h*128 + t//16 depends only on
head h.  Sharding: head-parallel over 8 cores (2 heads/core); every core
computes its two heads end-to-end and produces final-output rows
[256*i, 256*i+256).  Host concatenates -- no collectives.

fp8(e4m3) DoubleRow matmuls (2 packed k-values/PE-cell = 2x bf16
throughput; lhsT [128,2,M] / rhs [128,2,N], contraction 256) carry the QKV
projection and the PV/softmax-denominator matmuls, with a **bf16 island
for t<256 / s<256**: rows with concentrated causal attention (small t) are
the only places where fp8's ~3.6% element noise survives averaging, so
cols 0:256 of the first TQ=512 projection tile and the first two s-chunks
of every attention tile stay bf16 (island cols share one psum accumulation
group with the fp8 cols: the start matmul owns the bank, later start=False
matmuls zero their bytes on first write).  fp8 weights are pre-scaled x16
on the host (Wqkv values ~0.02 sit in the e4m3 subnormal range); the exp
scale SCALE/256 and Wout/16 cancel it exactly.  The out-projection and the
attention score matmuls stay bf16 (scores contract over d=128 only, so
DoubleRow cannot pair them anyway).  y is written bf16 and upcast on host.
Measured error: maxrel 7.5e-3 vs the 2e-2 gate.

Attention uses the S^T layout ([s,t]): softmax denominator via a ones
matmul (partition reduction on the PE), fp8-paired like PV.  Diagonal
128x128 triangle masks multiply the fp8 probs on gpsimd; for fp8 s-chunk
pairs whose two chunks start at different causal columns, the gap columns
of the later chunk are memset to 0 so the shared-width DoubleRow matmul
adds nothing there.

The TRN2 PE clock ramps (0.65 -> 1.2 -> 2.4 GHz) only under *continuous*
load; the schedule interleaves phases with independent tensor work:
    A: qkv(b=0)
    B: attention(b=0) + qkv(b=1)        (interleaved emission)
    C: attention(b=1) + out-proj(b=0)   (interleaved emission)
    D: out-proj(b=1)
RoPE is 3 DVE muls + 1 gpsimd add (sign baked into the sin table).
bf16<->fp8 PE config switches break ldweights pipelining, so attention
drains pop 2+ pending groups at once.  Phase D's Wout reload rides a
second weight pool (wo2) carved from the SBUF the qkv pools free after
phase B, prefetched during phase C.
"""

import math
import sys

for _p in ("/opt/trn_rl_repo", "/root/.axon_site/_ro/trn_rl_repo"):
    if _p not in sys.path:
        sys.path.insert(0, _p)

import numpy as np
import ml_dtypes

import concourse.bass as bass
import concourse.mybir as mybir
import concourse.tile as tile
from concourse import bacc
from concourse.bass_utils import run_bass_kernel_spmd

B, T, C = 2, 2048, 2048
H = 16            # heads total
D = C // H        # 128 head dim
HALF = D // 2     # 64
P = 128
KO = C // P       # 16 contraction chunks
NCORES = 8
HPC = H // NCORES  # 2 heads per core
TQ = 512          # t-tile for qkv projection
TISL = 256        # bf16 island width (t < TISL stays bf16)
NT = T // TQ
TA = 512          # t-tile for attention
NTA = T // TA
SCPT = TA // P    # 4 s-chunks per attention tile
TC_ = 512         # out-projection column tile
NCP = C // TC_
ROPE_BASE = 10000.0
SCALE = 1.0 / math.sqrt(D)
WS = 16.0         # fp8 weight pre-scale (host); exp scale / Wout absorb it

f32 = mybir.dt.float32
bf16 = mybir.dt.bfloat16
f8 = mybir.dt.float8e4
DR = mybir.MatmulPerfMode.DoubleRow


def _chain(*gens):
    for g in gens:
        yield from g


def _scale(gen, f):
    for v in gen:
        yield v * f


def _merge(*gens):
    """Cost-weighted round-robin: always step the generator with the least
    accumulated emitted-tensor-time.  Generators yield ns estimates."""
    acc = [0.0] * len(gens)
    live = list(range(len(gens)))
    while live:
        i = min(live, key=lambda k: acc[k])
        try:
            acc[i] += next(gens[i])
        except StopIteration:
            live.remove(i)


def _build():
    nc = bacc.Bacc("TRN2", target_bir_lowering=False, debug=False,
                   num_devices=NCORES)

    # host-pre-tiled x^T, fp8: xTt[b, ti, p, ko, u] = x[b, ti*TQ+u, ko*128+p]
    xTt = nc.dram_tensor("xTt", [B, NT, P, KO, TQ], f8, kind="ExternalInput")
    # bf16 island copy of the first ti tile (t < 256)
    xI = nc.dram_tensor("xI", [B, P, KO, TISL], bf16, kind="ExternalInput")
    # host-pre-chunked weights (x WS): wq/wk[p, hh, ko, d] = W[ko*128+p, hh*128+d]
    wq = nc.dram_tensor("wq", [P, HPC, KO, D], bf16, kind="ExternalInput")
    wk = nc.dram_tensor("wk", [P, HPC, KO, D], bf16, kind="ExternalInput")
    wv = nc.dram_tensor("wv", [P, KO, HPC * D], bf16, kind="ExternalInput")
    wq8 = nc.dram_tensor("wq8", [P, HPC, KO, D], f8, kind="ExternalInput")
    wk8 = nc.dram_tensor("wk8", [P, HPC, KO, D], f8, kind="ExternalInput")
    wv8 = nc.dram_tensor("wv8", [P, KO, HPC * D], f8, kind="ExternalInput")
    # woutp[cpi, p, j, m] = (Wout/WS)[j*128+p, cpi*TC_+m], bf16
    woutp = nc.dram_tensor("woutp", [NCP, P, KO, TC_], bf16,
                           kind="ExternalInput")
    cs2 = nc.dram_tensor("cs2", [P, T], bf16, kind="ExternalInput")  # [cos;cos]
    # sign-baked sin: snS[0:64] = -sin, snS[64:128] = +sin
    snS = nc.dram_tensor("snS", [P, T], bf16, kind="ExternalInput")
    # tri[s, u] = 1 iff s <= u (valid upper triangle in the S^T layout)
    tri = nc.dram_tensor("tri", [P, P], bf16, kind="ExternalInput")
    tri8 = nc.dram_tensor("tri8", [P, P], f8, kind="ExternalInput")
    y = nc.dram_tensor("y", [B, HPC * D, C], bf16, kind="ExternalOutput")

    with tile.TileContext(nc) as tc:
        with tc.tile_pool(name="const", bufs=1) as cp_, \
             tc.tile_pool(name="qkv", bufs=1) as qp, \
             tc.tile_pool(name="ot", bufs=1) as op_, \
             tc.tile_pool(name="wo", bufs=2) as wop, \
             tc.tile_pool(name="small", bufs=3) as sp, \
             tc.tile_pool(name="pt", bufs=6) as ptp, \
             tc.tile_pool(name="psBsc", bufs=2, space="PSUM") as pssc, \
             tc.tile_pool(name="psBo", bufs=1, space="PSUM") as pso, \
             tc.tile_pool(name="psA", bufs=2, space="PSUM") as psa, \
             tc.tile_pool(name="psBsum", bufs=1, space="PSUM") as pssum:

            wq_sb = cp_.tile([P, HPC, KO, D], bf16, tag="wq")
            wk_sb = cp_.tile([P, HPC, KO, D], bf16, tag="wk")
            wv_sb = cp_.tile([P, KO, HPC * D], bf16, tag="wv")
            wq8_sb = cp_.tile([P, HPC, KO, D], f8, tag="wq8")
            wk8_sb = cp_.tile([P, HPC, KO, D], f8, tag="wk8")
            wv8_sb = cp_.tile([P, KO, HPC * D], f8, tag="wv8")
            # first q matmuls need only wq[:, 0]; split the DMA so they
            # start sooner.  cs/sn head chunks come first: RoPE(ti=0) gates
            # the psum-accumulator recycling.
            nc.sync.dma_start(wq_sb[:, 0], wq.ap()[:, 0])
            cs_sb = cp_.tile([P, T], bf16, tag="cs")
            sn_sb = cp_.tile([P, T], bf16, tag="sn")
            tri_sb = cp_.tile([P, P], bf16, tag="tri")
            tri8_sb = cp_.tile([P, P], f8, tag="tri8")
            # ones *matrix* stationary for the denominator matmuls (bf16 for
            # the island chunks, fp8 k-pair for the DoubleRow chunks): cost ~
            # moving size, keeps the PE tile config at (128,128), and
            # broadcasts the sums to all partitions so normalization is a
            # plain elementwise multiply.
            # PE warmup: the clock ramp (0.65->2.4GHz) needs sustained
            # activity; burn cheap dummy matmuls while the first DMAs land.
            # gpsimd does the memset so the chain doesn't wait on DVE init.
            warm_sb = cp_.tile([P, TA], bf16, tag="warm_sb")
            warm_st = cp_.tile([P, 1], bf16, tag="warm_st")
            nc.gpsimd.memset(warm_st[:], 0.0)
            nc.gpsimd.memset(warm_sb[:], 0.0)
            ps_warm = pssc.tile([P, TA], f32, tag="sc", name="warm")
            for wi in range(14):
                nc.tensor.matmul(ps_warm[0:1, :], warm_st[:], warm_sb[:],
                                 start=True, stop=True)

            ones_f32 = cp_.tile([P, P], f32, tag="ones_f32")
            nc.vector.memset(ones_f32[:], 1.0)
            ones_mat = cp_.tile([P, P], bf16, tag="ones_mat")
            nc.vector.tensor_copy(ones_mat[:], ones_f32[:])
            ones8 = cp_.tile([P, 2, P], f8, tag="ones8")
            nc.vector.memset(ones8[:], 1.0)

            # persistent attention outputs O^T per (b, local head): [d, t]
            oT = [[op_.tile([P, T], bf16, tag=f"oT{b}{hh}", name=f"oT{b}{hh}")
                   for hh in range(HPC)] for b in range(B)]
            qT = [[qp.tile([P, T], bf16, tag=f"qT{b}{hh}", name=f"qT{b}{hh}")
                   for hh in range(HPC)] for b in range(B)]
            kT = [[qp.tile([P, T], bf16, tag=f"kT{b}{hh}", name=f"kT{b}{hh}")
                   for hh in range(HPC)] for b in range(B)]
            # v chunks: s-chunks 0..1 bf16 (island), all chunks 2+ fp8
            vt16 = [[qp.tile([P, 1, D], bf16, tag=f"v16{b}{hh}",
                             name=f"v16{b}{hh}")
                     for hh in range(HPC)] for b in range(B)]
            vt8 = [[qp.tile([P, T // P, D], f8, tag=f"v8{b}{hh}",
                            name=f"v8{b}{hh}")
                    for hh in range(HPC)] for b in range(B)]

            def gen_qkv(b, xp, xpi, psa, rp, state, tis=None):
                for ti in (range(NT) if tis is None else tis):
                    sl = slice(ti * TQ, (ti + 1) * TQ)
                    first_tile = (b == 0 and ti == 0)
                    island = (ti == 0)
                    xi = None
                    if island:
                        if state.get("nextI") is not None:
                            xi, xt = state.pop("nextI")
                        else:
                            xi = xpi.tile([P, KO, TISL], bf16, tag="xti",
                                          name=f"xti{b}")
                            xt = xpi.tile([P, KO, TISL], f8, tag="xt0",
                                          name=f"xt0{b}")
                            if not first_tile:
                                nc.sync.dma_start(xi[:], xI.ap()[b])
                                nc.sync.dma_start(
                                    xt[:], xTt.ap()[b, 0, :, :, TISL:TQ])
                    elif state.get("next") is not None:
                        xt = state.pop("next")
                    else:
                        xt = xp.tile([P, KO, TQ], f8, tag="xt",
                                     name=f"xt{b}_{ti}")
                        nc.sync.dma_start(xt[:], xTt.ap()[b, ti])

                    if not first_tile:
                        # prefetch the NEXT section's x right away: the DMA
                        # queues are idle at a section boundary, and the full
                        # section (~10us of PE work) hides the transfer.
                        if ti + 1 < NT:
                            if (b, ti + 1) > state.get("pref", (-1, -1)):
                                xtn = xp.tile([P, KO, TQ], f8, tag="xt",
                                              name=f"xt{b}_{ti + 1}")
                                nc.sync.dma_start(xtn[:], xTt.ap()[b, ti + 1])
                                state["next"] = xtn
                                state["pref"] = (b, ti + 1)
                        elif b + 1 < B and (b + 1, 0) > state.get(
                                "pref", (-1, -1)):
                            xi2 = xpi.tile([P, KO, TISL], bf16, tag="xti",
                                           name=f"xti{b + 1}")
                            xt2 = xpi.tile([P, KO, TISL], f8, tag="xt0",
                                           name=f"xt0{b + 1}")
                            nc.sync.dma_start(xi2[:], xI.ap()[b + 1])
                            nc.sync.dma_start(
                                xt2[:], xTt.ap()[b + 1, 0, :, :, TISL:TQ])
                            state["nextI"] = (xi2, xt2)
                            state["pref"] = (b + 1, 0)

                    cs = cs_sb[:, sl]
                    sn = sn_sb[:, sl]

                    def qkmm(w_sb, w8_sb, hh, split_dma=False, csn_after=None):
                        ps = psa.tile([P, TQ], f32, tag="acc",
                                      name=f"acc{b}_{ti}_{hh}")
                        if island:
                            # ONE accumulation group per psum bank: bf16
                            # island cols 0:TISL carry the start flag; the
                            # fp8 pair matmuls on cols TISL:TQ zero their
                            # bytes on first write within the started bank.
                            for ko in range(KO):
                                if split_dma and ko % 4 == 0:
                                    kos = slice(ko, ko + 4)
                                    nc.sync.dma_start(xi[:, kos],
                                                      xI.ap()[b, :, kos])
                                    if ko == 4 and csn_after:
                                        nc.sync.dma_start(cs_sb[:, 0:TQ],
                                                          cs2.ap()[:, 0:TQ])
                                        nc.sync.dma_start(sn_sb[:, 0:TQ],
                                                          snS.ap()[:, 0:TQ])
                                nc.tensor.matmul(ps[:, 0:TISL],
                                                 w_sb[:, hh, ko, :],
                                                 xi[:, ko, :],
                                                 start=(ko == 0), stop=False)
                            if split_dma:
                                nc.sync.dma_start(
                                    xt[:], xTt.ap()[b, 0, :, :, TISL:TQ])
                            for kp in range(KO // 2):
                                nc.tensor.matmul(
                                    ps[:, TISL:TQ],
                                    w8_sb[:, hh, 2 * kp:2 * kp + 2, :],
                                    xt[:, 2 * kp:2 * kp + 2, :],
                                    start=False, stop=(kp == KO // 2 - 1),
                                    perf_mode=DR)
                        else:
                            for kp in range(KO // 2):
                                nc.tensor.matmul(
                                    ps[:], w8_sb[:, hh, 2 * kp:2 * kp + 2, :],
                                    xt[:, 2 * kp:2 * kp + 2, :],
                                    start=(kp == 0), stop=(kp == KO // 2 - 1),
                                    perf_mode=DR)
                        return ps

                    def rope(ps, dst):
                        # tcos = ps * [cos;cos]; tsw pre-swaps halves with
                        # the sign baked into snS (rows 0:64 hold -sin), so
                        # ONE full-width gpsimd add finishes the rotation.
                        tcos = rp.tile([P, TQ], bf16, tag="tcos")
                        tsw = rp.tile([P, TQ], bf16, tag="tsw")
                        nc.vector.tensor_mul(tcos[:], ps[:], cs)
                        nc.vector.tensor_mul(tsw[0:HALF, :],
                                             ps[HALF:P, :], sn[0:HALF, :])
                        nc.vector.tensor_mul(tsw[HALF:P, :],
                                             ps[0:HALF, :], sn[HALF:P, :])
                        nc.gpsimd.tensor_add(dst[:, sl], tcos[:], tsw[:])

                    if first_tile:
                        # q accums first (need only wq + xI quarters); the
                        # fp8 half of the island needs wq8[:, 0] -- tiny DMA
                        # issued up front.  Stagger the rest behind.
                        nc.sync.dma_start(wq8_sb[:, 0], wq8.ap()[:, 0])
                        ps0 = qkmm(wq_sb, wq8_sb, 0, split_dma=True,
                                   csn_after=1)
                        nc.sync.dma_start(wq_sb[:, 1], wq.ap()[:, 1])
                        nc.sync.dma_start(wq8_sb[:, 1], wq8.ap()[:, 1])
                        nc.sync.dma_start(wk_sb[:], wk.ap())
                        nc.sync.dma_start(wk8_sb[:], wk8.ap())
                        yield 2600.0
                        ps1 = qkmm(wq_sb, wq8_sb, 1)
                        nc.sync.dma_start(wv_sb[:], wv.ap())
                        nc.sync.dma_start(wv8_sb[:], wv8.ap())
                        xtn = xp.tile([P, KO, TQ], f8, tag="xt",
                                      name=f"xt{b}_1")
                        nc.sync.dma_start(xtn[:], xTt.ap()[b, 1])
                        state["next"] = xtn
                        state["pref"] = (b, 1)
                        rope(ps0, qT[b][0])
                        yield 2600.0
                        psk = qkmm(wk_sb, wk8_sb, 0)
                        nc.sync.dma_start(cs_sb[:, TQ:3 * TQ],
                                          cs2.ap()[:, TQ:3 * TQ])
                        nc.sync.dma_start(sn_sb[:, TQ:3 * TQ],
                                          snS.ap()[:, TQ:3 * TQ])
                        rope(ps1, qT[b][1])
                        rope(psk, kT[b][0])
                        yield 2600.0
                        psk = qkmm(wk_sb, wk8_sb, 1)
                        rope(psk, kT[b][1])
                        yield 2600.0
                    else:
                        for w_sb, w8_sb, dsts in ((wq_sb, wq8_sb, qT[b]),
                                                  (wk_sb, wk8_sb, kT[b])):
                            for hh in range(HPC):
                                rope(qkmm(w_sb, w8_sb, hh), dsts[hh])
                                yield 2600.0 if island else 1707.0
                    if b == 0 and ti == 1:
                        nc.sync.dma_start(cs_sb[:, 3 * TQ:],
                                          cs2.ap()[:, 3 * TQ:])
                        nc.sync.dma_start(sn_sb[:, 3 * TQ:],
                                          snS.ap()[:, 3 * TQ:])
                        nc.sync.dma_start(tri_sb[:], tri.ap())
                        nc.sync.dma_start(tri8_sb[:], tri8.ap())
                    # prefetch next x tiles before the v-section so their
                    # DMAs get ahead of lower-priority queue entries
                    if ti + 1 < NT:
                        if (b, ti + 1) > state.get("pref", (-1, -1)):
                            xtn = xp.tile([P, KO, TQ], f8, tag="xt",
                                          name=f"xt{b}_{ti + 1}")
                            nc.sync.dma_start(xtn[:], xTt.ap()[b, ti + 1])
                            state["next"] = xtn
                            state["pref"] = (b, ti + 1)
                    elif b + 1 < B and (b + 1, 0) > state.get("pref",
                                                              (-1, -1)):
                        xi2 = xpi.tile([P, KO, TISL], bf16, tag="xti",
                                       name=f"xti{b + 1}")
                        xt2 = xpi.tile([P, KO, TISL], f8, tag="xt0",
                                       name=f"xt0{b + 1}")
                        nc.sync.dma_start(xi2[:], xI.ap()[b + 1])
                        nc.sync.dma_start(
                            xt2[:], xTt.ap()[b + 1, 0, :, :, TISL:TQ])
                        state["nextI"] = (xi2, xt2)
                        state["pref"] = (b + 1, 0)
                    for sub in range(TQ // P):
                        tci = ti * (TQ // P) + sub
                        psvt = psa.tile([P, TQ], f32, tag="acc",
                                        name=f"accv{b}_{ti}_{sub}")
                        psv = psvt[:, 0:HPC * D]
                        if tci < 2:
                            for ko in range(KO):
                                nc.tensor.matmul(
                                    psv, xi[:, ko, sub * P:(sub + 1) * P],
                                    wv_sb[:, ko, :],
                                    start=(ko == 0), stop=(ko == KO - 1))
                        elif island:
                            for kp in range(KO // 2):
                                nc.tensor.matmul(
                                    psv,
                                    xt[:, 2 * kp:2 * kp + 2,
                                       (sub - 2) * P:(sub - 1) * P],
                                    wv8_sb[:, 2 * kp:2 * kp + 2, :],
                                    start=(kp == 0),
                                    stop=(kp == KO // 2 - 1),
                                    perf_mode=DR)
                        else:
                            for kp in range(KO // 2):
                                nc.tensor.matmul(
                                    psv,
                                    xt[:, 2 * kp:2 * kp + 2,
                                       sub * P:(sub + 1) * P],
                                    wv8_sb[:, 2 * kp:2 * kp + 2, :],
                                    start=(kp == 0),
                                    stop=(kp == KO // 2 - 1),
                                    perf_mode=DR)
                        for hh in range(HPC):
                            if tci < 1:
                                nc.vector.tensor_copy(
                                    vt16[b][hh][:, tci, :],
                                    psv[:, hh * D:(hh + 1) * D])
                            else:
                                nc.vector.tensor_copy(
                                    vt8[b][hh][:, tci, :],
                                    psv[:, hh * D:(hh + 1) * D])
                        yield 1707.0 if tci < 2 else 853.0

            def gen_attn(b):
                # Both heads interleaved; o/sum matmuls trail score/exp so
                # the PE isn't chained to the Exp latency.  Mask/normalize
                # run on gpsimd.  s-chunks 0,1 are the bf16 island; chunks
                # 2..smax go in fp8 DoubleRow pairs.
                for ta in range(NTA):
                    ps_o = [pso.tile([P, TA], f32, tag=f"o{hh}",
                                     name=f"o{b}_{ta}_{hh}")
                            for hh in range(HPC)]
                    ps_sum = pssum.tile([P, HPC, TA], f32, tag="sum",
                                        name=f"sum{b}_{ta}")
                    smax = (ta + 1) * SCPT - 1
                    # schedule: ('b16', s) singles for s=0,1; ('f8', s)
                    # pairs (s, s+1) for s=2,4,..,smax-1
                    sched = [("b16", 0)] + \
                        [("f8", s) for s in range(1, smax - 1, 2)] + \
                        [("f8s", smax)]
                    pending = []

                    def drain(n, _p=pending, _o=ps_o, _s=ps_sum):
                        while len(_p) > n:
                            kind, s, w, pt2, first, last = _p.pop(0)
                            if kind == "b16":
                                for hh in range(HPC):
                                    nc.tensor.matmul(_o[hh][:, w],
                                                     vt16[b][hh][:, s, :],
                                                     pt2[:, 0, hh, w],
                                                     start=first, stop=last)
                                    nc.tensor.matmul(_s[:, hh, w],
                                                     ones_mat[:],
                                                     pt2[:, 0, hh, w],
                                                     start=first, stop=last)
                            elif kind == "f8s":
                                for hh in range(HPC):
                                    nc.tensor.matmul(_o[hh][:, w],
                                                     vt8[b][hh][:, s, :],
                                                     pt2[:, 0, hh, w],
                                                     start=first, stop=last)
                                    nc.tensor.matmul(_s[:, hh, w],
                                                     ones8[:, 0, :],
                                                     pt2[:, 0, hh, w],
                                                     start=first, stop=last)
                            else:
                                for hh in range(HPC):
                                    nc.tensor.matmul(
                                        _o[hh][:, w],
                                        vt8[b][hh][:, s:s + 2, :],
                                        pt2[:, :, hh, w],
                                        start=first, stop=last, perf_mode=DR)
                                    nc.tensor.matmul(
                                        _s[:, hh, w], ones8[:],
                                        pt2[:, :, hh, w],
                                        start=first, stop=last, perf_mode=DR)

                    for gi, (kind, s) in enumerate(sched):
                        glen = 2 if kind == "f8" else 1
                        j0 = s - ta * SCPT
                        w0g = P * max(j0, 0)
                        wg = slice(w0g, TA)
                        # f8s singles borrow the f8 pair ring (slot 0 only)
                        pt2 = ptp.tile([P, 1 if kind == "b16" else 2,
                                        HPC, TA],
                                       bf16 if kind == "b16" else f8,
                                       tag="ptb16" if kind == "b16"
                                       else "ptf8",
                                       bufs=2 if kind == "b16" else None,
                                       name=f"pt{b}_{ta}_{s}")
                        cost = 0.0
                        for idx in range(glen):
                            sc_ = s + idx
                            j = sc_ - ta * SCPT
                            w0s = P * max(j, 0)
                            ws = slice(w0s, TA)
                            qsl = slice(ta * TA + w0s, (ta + 1) * TA)
                            if idx == 1 and w0s > w0g:
                                # DoubleRow pair shares width w0g; zero the
                                # later chunk's not-yet-valid columns
                                nc.gpsimd.memset(
                                    pt2[:, 1, :, w0g:w0s], 0.0)
                            for hh in range(HPC):
                                ps_sc = pssc.tile(
                                    [P, TA], f32, tag="sc",
                                    name=f"sc{b}_{ta}_{sc_}_{hh}")
                                nc.tensor.matmul(
                                    ps_sc[:, ws],
                                    kT[b][hh][:, sc_ * P:(sc_ + 1) * P],
                                    qT[b][hh][:, qsl],
                                    start=True, stop=True)
                                nc.scalar.activation(
                                    pt2[:, idx, hh, ws], ps_sc[:, ws],
                                    mybir.ActivationFunctionType.Exp,
                                    scale=SCALE / (WS * WS))
                                if j >= 0:  # mask the 128x128 triangle
                                    trm = tri_sb if kind == "b16" else tri8_sb
                                    nc.gpsimd.tensor_mul(
                                        pt2[:, idx, hh, w0s:w0s + P],
                                        pt2[:, idx, hh, w0s:w0s + P],
                                        trm[:])
                                cost += (TA - w0s) * 0.42
                        pending.append((kind, s, wg, pt2, gi == 0,
                                        gi == len(sched) - 1))
                        if len(pending) > 3:
                            drain(1)  # pop 2+ at once: fewer PE dtype switches
                        
                        # score (bf16) + pv/sum (fp8 pairs ~half;
                        # b16/f8s singles full rate)
                        yield cost * (2.0 if kind == "f8" else 3.0)
                    drain(0)
                    for hh in range(HPC):
                        recf = sp.tile([P, TA], f32, tag="recf")
                        nc.vector.reciprocal_approx_fast(
                            recf[:], ps_sum[:, hh, :])
                        o_sb = sp.tile([P, TA], f32, tag="o_sb")
                        nc.vector.tensor_copy(o_sb[:], ps_o[hh][:])
                        # write oT pre-shuffled for the out-projection:
                        # oT[p, j*128+u] = O^T[p, t=u*16+j]
                        oview = oT[b][hh].rearrange(
                            "p (j u) -> p u j", j=KO)[
                            :, (TA // 16) * ta:(TA // 16) * (ta + 1), :]
                        nc.gpsimd.tensor_mul(
                            oview,
                            o_sb[:].rearrange("p (u j) -> p u j", j=KO),
                            recf[:].rearrange("p (u j) -> p u j", j=KO))
                    yield 500.0

            def wload(b, cpi, pool=None):
                wcp = (pool or wop).tile([P, KO, TC_], bf16, tag="w",
                                         name=f"w{b}_{cpi}")
                nc.sync.dma_start(wcp[:], woutp.ap()[cpi])
                state_w[(b, cpi)] = wcp

            def gen_out(b, psc, cpis=None):
                for cpi in (cpis if cpis is not None else range(NCP)):
                    csl = slice(cpi * TC_, (cpi + 1) * TC_)
                    if (b, cpi) not in state_w:
                        wload(b, cpi)
                    wcp = state_w[(b, cpi)]
                    if b == 0 and cpi == 1:
                        # b=1's first two Wout tiles ride phase C's DMA slack
                        wload(1, 0, state_w["wop2"])
                        wload(1, 1, state_w["wop2"])
                    if b == 1 and cpi == 0:
                        wload(1, 2)
                        wload(1, 3)
                    nb_, ncpi = (b, cpi + 1) if cpi + 1 < NCP else (b + 1, 0)
                    if nb_ < B and (nb_, ncpi) not in state_w:
                        wload(nb_, ncpi)  # 1-ahead prefetch (wo ring: 2 bufs)
                    for hh in range(HPC):
                        psy = psc.tile([P, TC_], f32, tag="acc",
                                       name=f"y{b}_{cpi}_{hh}")
                        for j in range(KO):
                            nc.tensor.matmul(psy[:],
                                             oT[b][hh][:, j * P:(j + 1) * P],
                                             wcp[:, j, :],
                                             start=(j == 0),
                                             stop=(j == KO - 1))
                        ysb = sp.tile([P, TC_], bf16, tag="ysb")
                        if b == 1:
                            nc.scalar.copy(ysb[:], psy[:])
                        else:
                            nc.vector.tensor_copy(ysb[:], psy[:])
                        nc.sync.dma_start(
                            y.ap()[b, hh * D:(hh + 1) * D, csl], ysb[:])
                        yield 6800.0 if b == 0 else 3414.0

            state_w = {}
            qstate = {}
            with tc.tile_pool(name="xt", bufs=3) as xp, \
                 tc.tile_pool(name="xti", bufs=1) as xpi, \
                 tc.tile_pool(name="rope", bufs=2) as rp:
                # phase A: qkv(b=0) alone
                for _ in gen_qkv(0, xp, xpi, psa, rp, qstate):
                    pass
                # phase B: attention(b=0) interleaved with qkv(b=1)
                _merge(gen_attn(0), gen_qkv(1, xp, xpi, psa, rp, qstate))
                wload(0, 0)
                wload(0, 1)
            # SBUF freed by xt/xti/rope is reused to double-buffer the b=1
            # Wout tiles during phase C, so phase D never waits on DMA.
            with tc.tile_pool(name="wo2", bufs=2) as wop2:
                state_w["wop2"] = wop2
                # phase C: attention(b=1) interleaved with out-proj(b=0);
                # the last b=0 column tile bridges the C->D transition
                # while oT[b=1] finishes normalizing.
                _merge(gen_attn(1), gen_out(0, psa, range(NCP - 1)))
                for _ in gen_out(0, psa, range(NCP - 1, NCP)):
                    pass
                # phase D: out-proj(b=1)
                for _ in gen_out(1, psa):
                    pass

    nc.compile()
    return nc


_NC = None


def _get_nc():
    global _NC
    if _NC is None:
        _NC = _build()
    return _NC


def _host_tables():
    pos = np.arange(T, dtype=np.float32)[:, None]
    div = np.exp(np.arange(0, 2 * HALF, 2, dtype=np.float32)
                 * np.float32(-math.log(ROPE_BASE) / (2 * HALF)))
    ang = pos * div[None, :]
    cosv = np.cos(ang).astype(np.float32)   # [T, HALF]
    sinv = np.sin(ang).astype(np.float32)
    cosT = np.ascontiguousarray(cosv.T)     # [HALF, T]
    sinT = np.ascontiguousarray(sinv.T)
    cs2 = np.ascontiguousarray(np.concatenate([cosT, cosT], axis=0)
                           .astype(ml_dtypes.bfloat16))  # [P, T]
    snS = np.ascontiguousarray(np.concatenate([-sinT, sinT], axis=0)
                               .astype(ml_dtypes.bfloat16))  # [P, T]
    # triangle mask tri[s, u] = 1 iff s <= u
    uu = np.arange(P)[None, :]
    ss = np.arange(P)[:, None]
    trim = (ss <= uu).astype(ml_dtypes.bfloat16)
    tri8m = (ss <= uu).astype(ml_dtypes.float8_e4m3)
    return cs2, snS, trim, tri8m


def _make_in_maps(x, Wqkv, Wout):
    x = np.asarray(x, dtype=np.float32)
    Wqkv = np.asarray(Wqkv, dtype=np.float32)
    Wout = np.asarray(Wout, dtype=np.float32)
    assert x.shape == (B, T, C) and Wqkv.shape == (C, 3 * C) \
        and Wout.shape == (C, C)

    cs2, snS, trim, tri8m = _host_tables()
    # xTt[b, ti, p, ko, u] = x[b, ti*TQ+u, ko*128+p]
    xTr = x.reshape(B, NT, TQ, KO, P).transpose(0, 1, 4, 3, 2)
    xTt = np.ascontiguousarray(xTr.astype(ml_dtypes.float8_e4m3))
    xI = np.ascontiguousarray(xTr[:, 0, :, :, 0:TISL]
                              .astype(ml_dtypes.bfloat16))
    # woutp[cpi, p, j, m] = (Wout/WS)[j*128+p, cpi*TC_+m]
    woutp = np.ascontiguousarray(
        (Wout * np.float32(1.0 / WS)).astype(ml_dtypes.bfloat16)
        .reshape(KO, P, NCP, TC_).transpose(2, 1, 0, 3))

    in_maps = []
    for core in range(NCORES):
        h0 = core * HPC
        cols = slice(h0 * D, (h0 + HPC) * D)
        ws16, ws8 = [], []
        for part in range(3):
            w = Wqkv[:, part * C:(part + 1) * C][:, cols] * np.float32(WS)
            if part < 2:  # wq/wk: [P, HPC, KO, D]
                wr = np.ascontiguousarray(
                    w.reshape(KO, P, HPC, D).transpose(1, 2, 0, 3))
            else:         # wv: [P, KO, HPC*D]
                wr = np.ascontiguousarray(
                    w.reshape(KO, P, HPC * D).transpose(1, 0, 2))
            ws16.append(wr.astype(ml_dtypes.bfloat16))
            ws8.append(wr.astype(ml_dtypes.float8_e4m3))
        in_maps.append({
            "xTt": xTt, "xI": xI,
            "wq": ws16[0], "wk": ws16[1], "wv": ws16[2],
            "wq8": ws8[0], "wk8": ws8[1], "wv8": ws8[2],
            "woutp": woutp,
            "cs2": cs2, "snS": snS, "tri": trim, "tri8": tri8m,
        })
    return in_maps


def _run(x, Wqkv, Wout, trace=False):
    nc = _get_nc()
    in_maps = _make_in_maps(x, Wqkv, Wout)
    res = run_bass_kernel_spmd(nc, in_maps, core_ids=list(range(NCORES)),
                               trace=trace)
    out = np.empty((B, T, C), dtype=np.float32)
    for core in range(NCORES):
        out[:, core * HPC * D:(core + 1) * HPC * D, :] = \
            res.results[core]["y"].astype(np.float32)
    return out, res


def kernel(x, Wqkv, Wout):
    out, _ = _run(x, Wqkv, Wout)
    return out
